# revision 8
# baseline (speedup 1.0000x reference)
"""GCN link predictor on 8 TRN2 NeuronCores (Bass/Tile) — v2.

Design notes (driven by HW profiling of the v1 baseline, 2.34 ms):
the bottleneck is the GPSIMD (Pool) engine generating SWDGE descriptors for
dma_gather at ~8 ns/index, blocking, with no faster indexed primitive on the
machine (ap_gather ~27 ns/idx, scatter_add wedges the device).  So v2
minimizes gather indices and keeps everything else off the Pool engine:

- Normalization refactor: out[d] = b + dinv[d] * sum_e table[src_e] with
  table rows pre-scaled by dinv[src] (x rows host-scaled; q~ scaled dinv^2
  at evacuation).  Indicators become PURE one-hot -> single-op is_equal on
  DVE, and PSUM is seeded with outer(bias, sqrt(deg)) so the dst-side dinv
  folds into the existing evacuation scale.
- Self-loops never enter the gather path: one identity matmul per dst tile
  adds p~[d] (resp. q~[d]) from SBUF-resident local tiles.
- Decode: label pairs sharded by core(a); the a-side z rows come from PE
  one-hot selection out of SBUF-resident local z tiles (overlaps the last
  AllGather); only the b-side uses dma_gather (transpose=True -> zbT
  [ch, pair]); dot product = DVE multiply + PE ones-reduction.

dma_gather indices are int16, so 40000-row tables are addressed through two
overlapping views: A = rows [0, 32768), B = rows [7232, 40000).
"""

import numpy as np

import concourse.bass as bass
import concourse.bacc as bacc
import concourse.mybir as mybir
import concourse.tile as tile
from concourse.bass_utils import run_bass_kernel_spmd

P = 128
N_NODES = 40000
IN_CH = 128
HID_CH = 128
OUT_CH = 64
N_LABEL = 200000
N_CORES = 8
NPC = N_NODES // N_CORES          # 5000 nodes per core
NT = (NPC + P - 1) // P           # 40 dst tiles per core (last has 8 nodes)
A_LIM = 32768                     # view A = rows [0, 32768)
B_OFF = N_NODES - A_LIM           # 7232; view B = rows [7232, 40000)
GROUP = 8                         # dst tiles per gather group
LBATCH = 32                       # decode chunks per gather batch

F16 = mybir.dt.float16
F32 = mybir.dt.float32
I16 = mybir.dt.int16


def _wrap16(flat):
    """dma_gather / index SBUF image: position n -> [n % 16, n // 16],
    replicated across the 8 groups of 16 partitions. [128, len/16] int16."""
    n = len(flat)
    assert n % 16 == 0
    grid = np.asarray(flat, np.int16).reshape(n // 16, 16).T
    return np.tile(grid, (8, 1))


def _prepare(x, edge_index, edge_label_index, W1, b1, W2, b2):
    src = np.asarray(edge_index[0], np.int64)
    dst = np.asarray(edge_index[1], np.int64)
    deg = (np.bincount(dst, minlength=N_NODES) + 1).astype(np.float64)
    dinv = (1.0 / np.sqrt(deg)).astype(np.float32)
    sqrtdeg = np.sqrt(deg).astype(np.float32)

    # ---- bucket edges by (core, tile), sorted by src within each bucket
    core_of = dst // NPC
    tloc = (dst % NPC) // P
    order = np.lexsort((src, tloc, core_of))
    s_src = src[order]
    s_dst = dst[order]
    key = core_of[order] * NT + tloc[order]
    starts = np.searchsorted(key, np.arange(N_CORES * NT))
    ends = np.searchsorted(key, np.arange(N_CORES * NT) + 1)

    cnt = (ends - starts).reshape(N_CORES, NT)
    fA = np.empty((N_CORES, NT), np.int64)   # forced-A (< B_OFF)
    fB = np.empty((N_CORES, NT), np.int64)   # forced-B (>= A_LIM)
    for k in range(N_CORES):
        for t in range(NT):
            b = k * NT + t
            ss = s_src[starts[b]:ends[b]]
            fA[k, t] = np.searchsorted(ss, B_OFF)
            fB[k, t] = len(ss) - np.searchsorted(ss, A_LIM)
    NCA = np.maximum(1, (fA.max(axis=0) + P - 1) // P)          # [NT]
    nA = np.minimum(cnt - fB, P * NCA[None, :])                 # [cores, NT]
    nA = np.maximum(nA, 0)
    cntB = cnt - nA
    NCB = (cntB.max(axis=0) + P - 1) // P                       # [NT]

    groups = []
    tile_chunks = {}
    gbase = 0
    col = 0
    for g0 in range(0, NT, GROUP):
        ts = list(range(g0, min(g0 + GROUP, NT)))
        gnA = int(NCA[ts].sum())
        gnB = int(NCB[ts].sum())
        groups.append(dict(tiles=ts, base=gbase, nA=gnA, nB=gnB,
                           colA=col, colB=col + gnA * 8))
        ca = gbase
        cb = gbase + gnA
        for t in ts:
            tile_chunks[t] = (list(range(ca, ca + int(NCA[t])))
                              + list(range(cb, cb + int(NCB[t]))))
            ca += int(NCA[t])
            cb += int(NCB[t])
        gbase += gnA + gnB
        col += (gnA + gnB) * 8
    TOT_CH = gbase
    WC = col

    # ---- per-core conv arrays: gather indices + dst-local one-hot columns
    cores = []
    for k in range(N_CORES):
        eidx = np.zeros((P, WC), np.int16)
        edloc = np.full((P, TOT_CH), -1.0, np.float32)  # -1 -> no is_eq match
        for g in groups:
            flatA = []
            flatB = []
            for t in g["tiles"]:
                b = k * NT + t
                ss = s_src[starts[b]:ends[b]]
                dd = s_dst[starts[b]:ends[b]]
                na = int(nA[k, t])
                la = np.full(int(NCA[t]) * P, -1.0, np.float32)
                ia = np.zeros(int(NCA[t]) * P, np.int64)
                ia[:na] = ss[:na]
                la[:na] = (dd[:na] - k * NPC - t * P).astype(np.float32)
                lb_ = np.full(int(NCB[t]) * P, -1.0, np.float32)
                ib = np.zeros(int(NCB[t]) * P, np.int64)
                nb = int(cntB[k, t])
                ib[:nb] = ss[na:na + nb] - B_OFF
                lb_[:nb] = (dd[na:na + nb] - k * NPC - t * P).astype(
                    np.float32)
                flatA.append((ia, la))
                flatB.append((ib, lb_))
            ia = np.concatenate([f[0] for f in flatA])
            ib = np.concatenate([f[0] for f in flatB])
            locs = np.concatenate([f[1] for f in flatA]
                                  + [f[1] for f in flatB])
            if len(ia):
                eidx[:, g["colA"]:g["colA"] + len(ia) // 16] = _wrap16(ia)
            if len(ib):
                eidx[:, g["colB"]:g["colB"] + len(ib) // 16] = _wrap16(ib)
            nch = g["nA"] + g["nB"]
            edloc[:, g["base"]:g["base"] + nch] = locs.reshape(nch, P).T
        cores.append(dict(eidx=eidx, edloc=edloc))

    # ---- decode prep: pairs sharded by core(a), grouped by (a_tile, b_view)
    la_all = np.asarray(edge_label_index[0], np.int64)
    lb_all = np.asarray(edge_label_index[1], np.int64)
    owner = la_all // NPC
    # per (core, a_tile, view) counts to find shared padded chunk counts
    atile = (la_all % NPC) // P
    bview = (lb_all >= A_LIM).astype(np.int64)  # 0 -> A view, 1 -> B view
    cntd = np.zeros((N_CORES, NT, 2), np.int64)
    for k in range(N_CORES):
        m = owner == k
        np.add.at(cntd[k], (atile[m], bview[m]), 1)
    NCD = (cntd.max(axis=0) + P - 1) // P                       # [NT, 2]
    # chunk layout: all view-A chunks (tile-major), then all view-B chunks
    chunksA = []
    chunksB = []
    for t in range(NT):
        for c in range(int(NCD[t, 0])):
            chunksA.append(t)
        for c in range(int(NCD[t, 1])):
            chunksB.append(t)
    LCH = len(chunksA) + len(chunksB)
    lbatches = []
    for v, chs, base in ((0, chunksA, 0), (1, chunksB, len(chunksA))):
        for c0 in range(0, len(chs), LBATCH):
            nch = min(LBATCH, len(chs) - c0)
            lbatches.append(dict(view=v, base=base + c0, nch=nch,
                                 tiles=chs[c0:c0 + nch]))
    WL = LCH * 8

    perms = []
    for k in range(N_CORES):
        m = owner == k
        ga, gb, gidx = la_all[m], lb_all[m], np.nonzero(m)[0]
        at, bv = atile[m], bview[m]
        o = np.lexsort((gb, bv, at))
        ga, gb, gidx, at, bv = ga[o], gb[o], gidx[o], at[o], bv[o]
        lidx = np.zeros((P, WL), np.int16)
        laloc = np.full(LCH * P, -1.0, np.float32)
        perm = np.full(LCH * P, -1, np.int64)
        cbase = {0: 0, 1: len(chunksA)}
        coff = {0: 0, 1: 0}
        for t in range(NT):
            for v in (0, 1):
                mm = (at == t) & (bv == v)
                pa, pb, pi = ga[mm], gb[mm], gidx[mm]
                ncap = int(NCD[t, v]) * P
                assert len(pa) <= ncap
                ids = np.zeros(ncap, np.int64)
                ids[:len(pb)] = pb - (0 if v == 0 else B_OFF)
                start = cbase[v] + coff[v]
                lidx[:, start * 8:(start + int(NCD[t, v])) * 8] = \
                    _wrap16(ids)
                sl = slice(start * P, start * P + len(pa))
                laloc[start * P:(start + int(NCD[t, v])) * P][:len(pa)] = \
                    (pa - k * NPC - t * P).astype(np.float32)
                perm[sl] = pi
                coff[v] += int(NCD[t, v])
        # laloc broadcast image: [128 partitions, LCH*128] fp16, value =
        # a_loc of the pair in that column (same in every partition)
        lab = np.broadcast_to(laloc[None, :], (P, LCH * P)).astype(np.float16)
        cores[k]["lidx"] = lidx
        cores[k]["laloc"] = np.ascontiguousarray(lab)
        perms.append(perm)

    # ---- dense inputs per core
    x = np.asarray(x, np.float32)
    for k in range(N_CORES):
        xk = x[k * NPC:(k + 1) * NPC] * dinv[k * NPC:(k + 1) * NPC, None]
        cores[k]["xT"] = np.ascontiguousarray(xk.T).astype(np.float16)
        cores[k]["W1h"] = np.asarray(W1, np.float32).astype(np.float16)
        cores[k]["W2h"] = np.asarray(W2, np.float32).astype(np.float16)
        cores[k]["b1row"] = np.asarray(b1, np.float32).astype(
            np.float16).reshape(1, HID_CH)
        cores[k]["b2row"] = np.asarray(b2, np.float32).astype(
            np.float16).reshape(1, OUT_CH)
        sq = np.zeros((1, NT * P), np.float16)
        sq[0, :NPC] = sqrtdeg[k * NPC:(k + 1) * NPC]
        cores[k]["sqrow"] = sq
        dk1 = np.ones((P, NT), np.float32)
        dk2 = np.ones((P, NT), np.float32)
        dv = dinv[k * NPC:(k + 1) * NPC]
        for t in range(NT):
            m = min(P, NPC - t * P)
            dk1[:m, t] = dv[t * P:t * P + m]
            dk2[:m, t] = dv[t * P:t * P + m] ** 2
        cores[k]["dk1"] = dk1
        cores[k]["dk2"] = dk2

    meta = dict(groups=groups, tile_chunks=tile_chunks, TOT_CH=TOT_CH,
                WC=WC, lbatches=lbatches, LCH=LCH, WL=WL,
                NCD=[[int(v) for v in row] for row in NCD])
    return meta, cores, perms


def _build(meta):
    TOT_CH, WC, LCH, WL = (meta["TOT_CH"], meta["WC"],
                           meta["LCH"], meta["WL"])
    NCHG_MAX = max(g["nA"] + g["nB"] for g in meta["groups"])

    nc = bacc.Bacc("TRN2", target_bir_lowering=False, debug=False,
                   num_devices=N_CORES)
    xT = nc.dram_tensor("xT", [P, NPC], F16, kind="ExternalInput")
    W1h = nc.dram_tensor("W1h", [P, HID_CH], F16, kind="ExternalInput")
    W2h = nc.dram_tensor("W2h", [P, OUT_CH], F16, kind="ExternalInput")
    b1row = nc.dram_tensor("b1row", [1, HID_CH], F16, kind="ExternalInput")
    b2row = nc.dram_tensor("b2row", [1, OUT_CH], F16, kind="ExternalInput")
    sqrow = nc.dram_tensor("sqrow", [1, NT * P], F16, kind="ExternalInput")
    dk1 = nc.dram_tensor("dk1", [P, NT], F32, kind="ExternalInput")
    dk2 = nc.dram_tensor("dk2", [P, NT], F32, kind="ExternalInput")
    eidx = nc.dram_tensor("eidx", [P, WC], I16, kind="ExternalInput")
    edloc = nc.dram_tensor("edloc", [P, TOT_CH], F32, kind="ExternalInput")
    lidx = nc.dram_tensor("lidx", [P, WL], I16, kind="ExternalInput")
    laloc = nc.dram_tensor("laloc", [P, LCH * P], F16, kind="ExternalInput")
    logits = nc.dram_tensor("logits", [P, LCH], F32, kind="ExternalOutput")

    RG = [list(range(N_CORES))]

    with tile.TileContext(nc) as tc:
        with tc.tile_pool(name="const", bufs=1) as cpool, \
             tc.tile_pool(name="msgp", bufs=2) as msgp, \
             tc.tile_pool(name="indp", bufs=4) as indp, \
             tc.tile_pool(name="evac", bufs=3) as evac, \
             tc.tile_pool(name="decp", bufs=2) as decp, \
             tc.tile_pool(name="psA", bufs=2, space="PSUM") as psA, \
             tc.tile_pool(name="psB", bufs=2, space="PSUM") as psB, \
             tc.tile_pool(name="dram", bufs=1, space="DRAM") as dram:

            # constants into SBUF
            xT_s = cpool.tile([P, NPC], F16)
            W1_s = cpool.tile([P, HID_CH], F16)
            W2_s = cpool.tile([P, OUT_CH], F16)
            b1_s = cpool.tile([1, HID_CH], F16)
            b2_s = cpool.tile([1, OUT_CH], F16)
            sq_s = cpool.tile([1, NT * P], F16)
            dk1_s = cpool.tile([P, NT], F32)
            dk2_s = cpool.tile([P, NT], F32)
            ei_s = cpool.tile([P, WC], I16)
            el_s = cpool.tile([P, TOT_CH], F32)
            li_s = cpool.tile([P, WL], I16)
            iota_s = cpool.tile([P, P], F16)
            pcol_s = cpool.tile([P, 1], F32)
            ident_s = cpool.tile([P, P], F16)
            ones_s = cpool.tile([P, 1], F16)
            p_keep = cpool.tile([P, NT, HID_CH], F16)
            q_keep = cpool.tile([P, NT, OUT_CH], F16)
            z_keep = cpool.tile([P, NT, OUT_CH], F16)
            logit_sb = cpool.tile([P, LCH], F32)
            nc.sync.dma_start(out=xT_s[:], in_=xT[:])
            nc.sync.dma_start(out=W1_s[:], in_=W1h[:])
            nc.sync.dma_start(out=W2_s[:], in_=W2h[:])
            nc.sync.dma_start(out=b1_s[:], in_=b1row[:])
            nc.sync.dma_start(out=b2_s[:], in_=b2row[:])
            nc.sync.dma_start(out=sq_s[:], in_=sqrow[:])
            nc.sync.dma_start(out=dk1_s[:], in_=dk1[:])
            nc.sync.dma_start(out=dk2_s[:], in_=dk2[:])
            nc.sync.dma_start(out=ei_s[:], in_=eidx[:])
            nc.sync.dma_start(out=el_s[:], in_=edloc[:])
            nc.sync.dma_start(out=li_s[:], in_=lidx[:])
            nc.vector.memset(ones_s[:], 1.0)
            nc.gpsimd.iota(iota_s[:], pattern=[[1, P]], base=0,
                           channel_multiplier=0,
                           allow_small_or_imprecise_dtypes=True)
            nc.gpsimd.iota(pcol_s[:], pattern=[[0, 1]], base=0,
                           channel_multiplier=1,
                           allow_small_or_imprecise_dtypes=True)
            pmat_s = cpool.tile([P, P], F16)
            nc.gpsimd.iota(pmat_s[:], pattern=[[0, P]], base=0,
                           channel_multiplier=1,
                           allow_small_or_imprecise_dtypes=True)
            nc.vector.tensor_scalar(
                out=ident_s[:], in0=iota_s[:], scalar1=pcol_s[:],
                scalar2=None, op0=mybir.AluOpType.is_equal)

            p_in = dram.tile([NPC, HID_CH], F16)
            PT = dram.tile([N_NODES, HID_CH], F16, addr_space="Shared")
            q_in = dram.tile([NPC, P], F16)
            QT = dram.tile([N_NODES, P], F16, addr_space="Shared")
            z_in = dram.tile([NPC, P], F16)
            ZT = dram.tile([N_NODES, P], F16, addr_space="Shared")

            # ---- stage 1: p~ = (x*dinv) @ W1, per tile; keep + publish
            for t in range(NT):
                m = min(P, NPC - t * P)
                psum_p = psB.tile([P, HID_CH], F32, tag="pp", space="PSUM")
                nc.tensor.matmul(out=psum_p[0:m, :],
                                 lhsT=xT_s[:, t * P:t * P + m],
                                 rhs=W1_s[:], start=True, stop=True)
                nc.scalar.copy(out=p_keep[0:m, t, :], in_=psum_p[0:m, :])
                nc.sync.dma_start(out=p_in[t * P:t * P + m, :],
                                  in_=p_keep[0:m, t, :])

            nc.gpsimd.collective_compute(
                "AllGather", mybir.AluOpType.bypass, replica_groups=RG,
                ins=[p_in.opt()], outs=[PT.opt()])

            def conv_layer(TBL, out_dram, is_conv1):
                for g in meta["groups"]:
                    nch = g["nA"] + g["nB"]
                    msg = msgp.tile([P, NCHG_MAX, P], F16, tag="msg")
                    if g["nA"]:
                        nc.gpsimd.dma_gather(
                            out_ap=msg[:, 0:g["nA"], :],
                            in_ap=TBL[0:A_LIM, :],
                            idxs_ap=ei_s[:, g["colA"]:g["colA"] + g["nA"] * 8],
                            num_idxs=g["nA"] * P, num_idxs_reg=g["nA"] * P,
                            elem_size=P, single_packet=False)
                    if g["nB"]:
                        nc.gpsimd.dma_gather(
                            out_ap=msg[:, g["nA"]:nch, :],
                            in_ap=TBL[B_OFF:N_NODES, :],
                            idxs_ap=ei_s[:, g["colB"]:g["colB"] + g["nB"] * 8],
                            num_idxs=g["nB"] * P, num_idxs_reg=g["nB"] * P,
                            elem_size=P, single_packet=False)
                    for t in g["tiles"]:
                        m = min(P, NPC - t * P)
                        chunks = meta["tile_chunks"][t]
                        if is_conv1:
                            # psum [ch, d], seeded outer(b1, sqrtdeg)
                            ps = psA.tile([HID_CH, P], F32, tag="agg1",
                                          space="PSUM")
                            nc.tensor.matmul(
                                out=ps[:, 0:m], lhsT=b1_s[:],
                                rhs=sq_s[:, t * P:t * P + m],
                                start=True, stop=False)
                        else:
                            # psum [d, ch], seeded outer(sqrtdeg, b2)
                            ps = psA.tile([P, OUT_CH], F32, tag="agg2",
                                          space="PSUM")
                            nc.tensor.matmul(
                                out=ps[0:m, :],
                                lhsT=sq_s[:, t * P:t * P + m],
                                rhs=b2_s[:], start=True, stop=False)
                        for gc in chunks:
                            lc = gc - g["base"]
                            ind = indp.tile([P, P], F16, tag="ind")
                            nc.vector.tensor_scalar(
                                out=ind[:], in0=iota_s[:],
                                scalar1=el_s[:, gc:gc + 1],
                                scalar2=None,
                                op0=mybir.AluOpType.is_equal)
                            if is_conv1:
                                nc.tensor.matmul(
                                    out=ps[:, 0:m], lhsT=msg[:, lc, :],
                                    rhs=ind[:, 0:m],
                                    start=False, stop=False)
                            else:
                                nc.tensor.matmul(
                                    out=ps[0:m, :], lhsT=ind[:, 0:m],
                                    rhs=msg[:, lc, 0:OUT_CH],
                                    start=False, stop=False)
                        # self-loop: += p~[d] (resp. q~[d]) via identity
                        if is_conv1:
                            nc.tensor.matmul(
                                out=ps[:, 0:m], lhsT=p_keep[0:m, t, :],
                                rhs=ident_s[0:m, 0:m],
                                start=False, stop=True)
                            hT = evac.tile([HID_CH, P], F16, tag="hT")
                            nc.scalar.activation(
                                out=hT[:, 0:m], in_=ps[:, 0:m],
                                func=mybir.ActivationFunctionType.Relu)
                            psq = psB.tile([P, HID_CH], F32, tag="pp",
                                           space="PSUM")
                            nc.tensor.matmul(out=psq[0:m, 0:OUT_CH],
                                             lhsT=hT[:, 0:m], rhs=W2_s[:],
                                             start=True, stop=True)
                            qsb = evac.tile([P, P], F16, tag="qev")
                            nc.vector.memset(qsb[:, OUT_CH:P], 0)
                            nc.scalar.activation(
                                out=qsb[0:m, 0:OUT_CH], in_=psq[0:m, 0:OUT_CH],
                                func=mybir.ActivationFunctionType.Copy,
                                scale=dk2_s[0:m, t:t + 1])  # psq slice
                            nc.scalar.copy(out=q_keep[0:m, t, :],
                                           in_=qsb[0:m, 0:OUT_CH])
                            nc.sync.dma_start(
                                out=out_dram[t * P:t * P + m, :],
                                in_=qsb[0:m, :])
                        else:
                            nc.tensor.matmul(
                                out=ps[0:m, :], lhsT=ident_s[0:m, 0:m],
                                rhs=q_keep[0:m, t, :],
                                start=False, stop=True)
                            zsb = evac.tile([P, P], F16, tag="qev")
                            nc.vector.memset(zsb[:, OUT_CH:P], 0)
                            nc.scalar.activation(
                                out=zsb[0:m, 0:OUT_CH], in_=ps[0:m, :],
                                func=mybir.ActivationFunctionType.Copy,
                                scale=dk1_s[0:m, t:t + 1])
                            nc.scalar.copy(out=z_keep[0:m, t, :],
                                           in_=zsb[0:m, 0:OUT_CH])
                            nc.sync.dma_start(
                                out=out_dram[t * P:t * P + m, :],
                                in_=zsb[0:m, :])

            conv_layer(PT, q_in, True)
            nc.gpsimd.collective_compute(
                "AllGather", mybir.AluOpType.bypass, replica_groups=RG,
                ins=[q_in.opt()], outs=[QT.opt()])
            conv_layer(QT, z_in, False)
            nc.gpsimd.collective_compute(
                "AllGather", mybir.AluOpType.bypass, replica_groups=RG,
                ins=[z_in.opt()], outs=[ZT.opt()])

            # ---- decode: za via PE selection from z_keep, zb via
            # transposed dma_gather from ZT; dot = DVE mult + PE reduce
            for b in meta["lbatches"]:
                nch = b["nch"]
                zbT = decp.tile([P, LBATCH * P], F16, tag="zbT")
                av = (0, A_LIM) if b["view"] == 0 else (B_OFF, N_NODES)
                nc.gpsimd.dma_gather(
                    out_ap=zbT[:, 0:nch * P].rearrange(
                        "p (a b) -> p a b", a=1),
                    in_ap=ZT[av[0]:av[1], :],
                    idxs_ap=li_s[:, b["base"] * 8:(b["base"] + nch) * 8],
                    num_idxs=nch * P, num_idxs_reg=nch * P,
                    elem_size=P, single_packet=False, transpose=True)
                # ACT firewall: don't let DVE read dma_gather-written SBUF
                zb2 = decp.tile([P, LBATCH * P], F16, tag="zb2")
                nc.scalar.copy(out=zb2[0:OUT_CH, 0:nch * P],
                               in_=zbT[0:OUT_CH, 0:nch * P])
                la_t = decp.tile([P, LBATCH * P], F16, tag="la")
                nc.sync.dma_start(
                    out=la_t[:, 0:nch * P],
                    in_=laloc[:, b["base"] * P:(b["base"] + nch) * P])
                for ci in range(nch):
                    t = b["tiles"][ci]
                    m = min(P, NPC - t * P)
                    sel = indp.tile([P, P], F16, tag="sel")
                    nc.vector.tensor_tensor(
                        out=sel[:], in0=pmat_s[:],
                        in1=la_t[:, ci * P:ci * P + P],
                        op=mybir.AluOpType.is_equal)
                    psa = psA.tile([HID_CH, P], F32, tag="agg1",
                                   space="PSUM")
                    nc.tensor.matmul(out=psa[0:OUT_CH, :],
                                     lhsT=z_keep[0:m, t, :],
                                     rhs=sel[0:m, :], start=True, stop=True)
                    scr = indp.tile([OUT_CH, P], F16, tag="scr")
                    nc.vector.tensor_tensor(
                        out=scr[:], in0=psa[0:OUT_CH, :],
                        in1=zb2[0:OUT_CH, ci * P:ci * P + P],
                        op=mybir.AluOpType.mult)
                    psl = psB.tile([P, HID_CH], F32, tag="pp",
                                   space="PSUM")
                    nc.tensor.matmul(out=psl[:, 0:1], lhsT=scr[:],
                                     rhs=ones_s[0:OUT_CH, :],
                                     start=True, stop=True)
                    cc = b["base"] + ci
                    nc.scalar.copy(out=logit_sb[:, cc:cc + 1], in_=psl[:, 0:1])
            nc.sync.dma_start(out=logits[:], in_=logit_sb[:])

    nc.compile()
    return nc


_CACHE = {}
TRACE = False          # set True (e.g. from test.py) to capture an NTFF trace
LAST_RESULT = None     # BassKernelResults of the most recent run


def kernel(**inputs):
    meta, cores, perms = _prepare(**inputs)
    key = (meta["TOT_CH"], meta["LCH"], meta["WC"], meta["WL"])
    if key not in _CACHE:
        _CACHE[key] = _build(meta)
    nc = _CACHE[key]
    names = ("xT", "W1h", "W2h", "b1row", "b2row", "sqrow", "dk1", "dk2",
             "eidx", "edloc", "lidx", "laloc")
    in_maps = [{n: c[n] for n in names} for c in cores]
    res = run_bass_kernel_spmd(nc, in_maps, core_ids=list(range(N_CORES)),
                               trace=TRACE)
    global LAST_RESULT
    LAST_RESULT = res
    out = np.empty(N_LABEL, np.float32)
    for k in range(N_CORES):
        vals = res.results[k]["logits"].T.ravel()
        perm = perms[k]
        m = perm >= 0
        out[perm[m]] = vals[m]
    return out


# revision 9
# speedup vs baseline: 1.1286x; 1.1286x over previous
"""GCN link predictor on 8 TRN2 NeuronCores (Bass/Tile) — v2.

Design notes (driven by HW profiling of the v1 baseline, 2.34 ms):
the bottleneck is the GPSIMD (Pool) engine generating SWDGE descriptors for
dma_gather at ~8 ns/index, blocking, with no faster indexed primitive on the
machine (ap_gather ~27 ns/idx, scatter_add wedges the device).  So v2
minimizes gather indices and keeps everything else off the Pool engine:

- Normalization refactor: out[d] = b + dinv[d] * sum_e table[src_e] with
  table rows pre-scaled by dinv[src] (x rows host-scaled; q~ scaled dinv^2
  at evacuation).  Indicators become PURE one-hot -> single-op is_equal on
  DVE, and PSUM is seeded with outer(bias, sqrt(deg)) so the dst-side dinv
  folds into the existing evacuation scale.
- Self-loops never enter the gather path: one identity matmul per dst tile
  adds p~[d] (resp. q~[d]) from SBUF-resident local tiles.
- Decode: label pairs sharded by core(a); the a-side z rows come from PE
  one-hot selection out of SBUF-resident local z tiles (overlaps the last
  AllGather); only the b-side uses dma_gather (transpose=True -> zbT
  [ch, pair]); dot product = DVE multiply + PE ones-reduction.

dma_gather indices are int16, so 40000-row tables are addressed through two
overlapping views: A = rows [0, 32768), B = rows [7232, 40000).
"""

import numpy as np

import concourse.bass as bass
import concourse.bacc as bacc
import concourse.mybir as mybir
import concourse.tile as tile
from concourse.bass_utils import run_bass_kernel_spmd

P = 128
N_NODES = 40000
IN_CH = 128
HID_CH = 128
OUT_CH = 64
N_LABEL = 200000
N_CORES = 8
NPC = N_NODES // N_CORES          # 5000 nodes per core
NT = (NPC + P - 1) // P           # 40 dst tiles per core (last has 8 nodes)
A_LIM = 32768                     # view A = rows [0, 32768)
B_OFF = N_NODES - A_LIM           # 7232; view B = rows [7232, 40000)
GROUP = 4                         # dst tiles per gather group
LBATCH = 32                       # decode chunks per gather batch

F16 = mybir.dt.float16
F32 = mybir.dt.float32
I16 = mybir.dt.int16


def _wrap16(flat):
    """dma_gather / index SBUF image: position n -> [n % 16, n // 16],
    replicated across the 8 groups of 16 partitions. [128, len/16] int16."""
    n = len(flat)
    assert n % 16 == 0
    grid = np.asarray(flat, np.int16).reshape(n // 16, 16).T
    return np.tile(grid, (8, 1))


def _prepare(x, edge_index, edge_label_index, W1, b1, W2, b2):
    src = np.asarray(edge_index[0], np.int64)
    dst = np.asarray(edge_index[1], np.int64)
    deg = (np.bincount(dst, minlength=N_NODES) + 1).astype(np.float64)
    dinv = (1.0 / np.sqrt(deg)).astype(np.float32)
    sqrtdeg = np.sqrt(deg).astype(np.float32)

    # ---- bucket edges by (core, tile), sorted by src within each bucket
    core_of = dst // NPC
    tloc = (dst % NPC) // P
    order = np.lexsort((src, tloc, core_of))
    s_src = src[order]
    s_dst = dst[order]
    key = core_of[order] * NT + tloc[order]
    starts = np.searchsorted(key, np.arange(N_CORES * NT))
    ends = np.searchsorted(key, np.arange(N_CORES * NT) + 1)

    cnt = (ends - starts).reshape(N_CORES, NT)
    fA = np.empty((N_CORES, NT), np.int64)   # forced-A (< B_OFF)
    fB = np.empty((N_CORES, NT), np.int64)   # forced-B (>= A_LIM)
    for k in range(N_CORES):
        for t in range(NT):
            b = k * NT + t
            ss = s_src[starts[b]:ends[b]]
            fA[k, t] = np.searchsorted(ss, B_OFF)
            fB[k, t] = len(ss) - np.searchsorted(ss, A_LIM)
    NCA = np.maximum(1, (fA.max(axis=0) + P - 1) // P)          # [NT]
    nA = np.minimum(cnt - fB, P * NCA[None, :])                 # [cores, NT]
    nA = np.maximum(nA, 0)
    cntB = cnt - nA
    NCB = (cntB.max(axis=0) + P - 1) // P                       # [NT]

    groups = []
    tile_chunks = {}
    gbase = 0
    col = 0
    for g0 in range(0, NT, GROUP):
        ts = list(range(g0, min(g0 + GROUP, NT)))
        gnA = int(NCA[ts].sum())
        gnB = int(NCB[ts].sum())
        groups.append(dict(tiles=ts, base=gbase, nA=gnA, nB=gnB,
                           colA=col, colB=col + gnA * 8))
        ca = gbase
        cb = gbase + gnA
        for t in ts:
            tile_chunks[t] = (list(range(ca, ca + int(NCA[t])))
                              + list(range(cb, cb + int(NCB[t]))))
            ca += int(NCA[t])
            cb += int(NCB[t])
        gbase += gnA + gnB
        col += (gnA + gnB) * 8
    TOT_CH = gbase
    WC = col

    # ---- per-core conv arrays: gather indices + dst-local one-hot columns
    cores = []
    for k in range(N_CORES):
        eidx = np.zeros((P, WC), np.int16)
        edloc = np.full((P, TOT_CH), -1.0, np.float32)  # -1 -> no is_eq match
        for g in groups:
            flatA = []
            flatB = []
            for t in g["tiles"]:
                b = k * NT + t
                ss = s_src[starts[b]:ends[b]]
                dd = s_dst[starts[b]:ends[b]]
                na = int(nA[k, t])
                la = np.full(int(NCA[t]) * P, -1.0, np.float32)
                ia = np.zeros(int(NCA[t]) * P, np.int64)
                ia[:na] = ss[:na]
                la[:na] = (dd[:na] - k * NPC - t * P).astype(np.float32)
                lb_ = np.full(int(NCB[t]) * P, -1.0, np.float32)
                ib = np.zeros(int(NCB[t]) * P, np.int64)
                nb = int(cntB[k, t])
                ib[:nb] = ss[na:na + nb] - B_OFF
                lb_[:nb] = (dd[na:na + nb] - k * NPC - t * P).astype(
                    np.float32)
                flatA.append((ia, la))
                flatB.append((ib, lb_))
            ia = np.concatenate([f[0] for f in flatA])
            ib = np.concatenate([f[0] for f in flatB])
            locs = np.concatenate([f[1] for f in flatA]
                                  + [f[1] for f in flatB])
            if len(ia):
                eidx[:, g["colA"]:g["colA"] + len(ia) // 16] = _wrap16(ia)
            if len(ib):
                eidx[:, g["colB"]:g["colB"] + len(ib) // 16] = _wrap16(ib)
            nch = g["nA"] + g["nB"]
            edloc[:, g["base"]:g["base"] + nch] = locs.reshape(nch, P).T
        cores.append(dict(eidx=eidx, edloc=edloc))

    # ---- decode prep: pairs sharded by core(a), grouped by (a_tile, b_view)
    la_all = np.asarray(edge_label_index[0], np.int64)
    lb_all = np.asarray(edge_label_index[1], np.int64)
    owner = la_all // NPC
    # per (core, a_tile, view) counts to find shared padded chunk counts
    atile = (la_all % NPC) // P
    bview = (lb_all >= A_LIM).astype(np.int64)  # 0 -> A view, 1 -> B view
    cntd = np.zeros((N_CORES, NT, 2), np.int64)
    for k in range(N_CORES):
        m = owner == k
        np.add.at(cntd[k], (atile[m], bview[m]), 1)
    NCD = (cntd.max(axis=0) + P - 1) // P                       # [NT, 2]
    # chunk layout: all view-A chunks (tile-major), then all view-B chunks
    chunksA = []
    chunksB = []
    for t in range(NT):
        for c in range(int(NCD[t, 0])):
            chunksA.append(t)
        for c in range(int(NCD[t, 1])):
            chunksB.append(t)
    LCH = len(chunksA) + len(chunksB)
    lbatches = []
    for v, chs, base in ((0, chunksA, 0), (1, chunksB, len(chunksA))):
        for c0 in range(0, len(chs), LBATCH):
            nch = min(LBATCH, len(chs) - c0)
            lbatches.append(dict(view=v, base=base + c0, nch=nch,
                                 tiles=chs[c0:c0 + nch]))
    WL = LCH * 8

    perms = []
    for k in range(N_CORES):
        m = owner == k
        ga, gb, gidx = la_all[m], lb_all[m], np.nonzero(m)[0]
        at, bv = atile[m], bview[m]
        o = np.lexsort((gb, bv, at))
        ga, gb, gidx, at, bv = ga[o], gb[o], gidx[o], at[o], bv[o]
        lidx = np.zeros((P, WL), np.int16)
        laloc = np.full(LCH * P, -1.0, np.float32)
        perm = np.full(LCH * P, -1, np.int64)
        cbase = {0: 0, 1: len(chunksA)}
        coff = {0: 0, 1: 0}
        for t in range(NT):
            for v in (0, 1):
                mm = (at == t) & (bv == v)
                pa, pb, pi = ga[mm], gb[mm], gidx[mm]
                ncap = int(NCD[t, v]) * P
                assert len(pa) <= ncap
                ids = np.zeros(ncap, np.int64)
                ids[:len(pb)] = pb - (0 if v == 0 else B_OFF)
                start = cbase[v] + coff[v]
                lidx[:, start * 8:(start + int(NCD[t, v])) * 8] = \
                    _wrap16(ids)
                sl = slice(start * P, start * P + len(pa))
                laloc[start * P:(start + int(NCD[t, v])) * P][:len(pa)] = \
                    (pa - k * NPC - t * P).astype(np.float32)
                perm[sl] = pi
                coff[v] += int(NCD[t, v])
        # laloc broadcast image: [128 partitions, LCH*128] fp16, value =
        # a_loc of the pair in that column (same in every partition)
        lab = np.broadcast_to(laloc[None, :], (P, LCH * P)).astype(np.float16)
        cores[k]["lidx"] = lidx
        cores[k]["laloc"] = np.ascontiguousarray(lab)
        perms.append(perm)

    # ---- dense inputs per core
    x = np.asarray(x, np.float32)
    for k in range(N_CORES):
        xk = x[k * NPC:(k + 1) * NPC] * dinv[k * NPC:(k + 1) * NPC, None]
        cores[k]["xT"] = np.ascontiguousarray(xk.T).astype(np.float16)
        cores[k]["W1h"] = np.asarray(W1, np.float32).astype(np.float16)
        cores[k]["W2h"] = np.asarray(W2, np.float32).astype(np.float16)
        cores[k]["b1row"] = np.asarray(b1, np.float32).astype(
            np.float16).reshape(1, HID_CH)
        cores[k]["b2row"] = np.asarray(b2, np.float32).astype(
            np.float16).reshape(1, OUT_CH)
        sq = np.zeros((1, NT * P), np.float16)
        sq[0, :NPC] = sqrtdeg[k * NPC:(k + 1) * NPC]
        cores[k]["sqrow"] = sq
        dk1 = np.ones((P, NT), np.float32)
        dk2 = np.ones((P, NT), np.float32)
        dv = dinv[k * NPC:(k + 1) * NPC]
        for t in range(NT):
            m = min(P, NPC - t * P)
            dk1[:m, t] = dv[t * P:t * P + m]
            dk2[:m, t] = dv[t * P:t * P + m] ** 2
        cores[k]["dk1"] = dk1
        cores[k]["dk2"] = dk2

    meta = dict(groups=groups, tile_chunks=tile_chunks, TOT_CH=TOT_CH,
                WC=WC, lbatches=lbatches, LCH=LCH, WL=WL,
                NCD=[[int(v) for v in row] for row in NCD])
    return meta, cores, perms


def _build(meta):
    TOT_CH, WC, LCH, WL = (meta["TOT_CH"], meta["WC"],
                           meta["LCH"], meta["WL"])
    NCHG_MAX = max(g["nA"] + g["nB"] for g in meta["groups"])

    nc = bacc.Bacc("TRN2", target_bir_lowering=False, debug=False,
                   num_devices=N_CORES)
    xT = nc.dram_tensor("xT", [P, NPC], F16, kind="ExternalInput")
    W1h = nc.dram_tensor("W1h", [P, HID_CH], F16, kind="ExternalInput")
    W2h = nc.dram_tensor("W2h", [P, OUT_CH], F16, kind="ExternalInput")
    b1row = nc.dram_tensor("b1row", [1, HID_CH], F16, kind="ExternalInput")
    b2row = nc.dram_tensor("b2row", [1, OUT_CH], F16, kind="ExternalInput")
    sqrow = nc.dram_tensor("sqrow", [1, NT * P], F16, kind="ExternalInput")
    dk1 = nc.dram_tensor("dk1", [P, NT], F32, kind="ExternalInput")
    dk2 = nc.dram_tensor("dk2", [P, NT], F32, kind="ExternalInput")
    eidx = nc.dram_tensor("eidx", [P, WC], I16, kind="ExternalInput")
    edloc = nc.dram_tensor("edloc", [P, TOT_CH], F32, kind="ExternalInput")
    lidx = nc.dram_tensor("lidx", [P, WL], I16, kind="ExternalInput")
    laloc = nc.dram_tensor("laloc", [P, LCH * P], F16, kind="ExternalInput")
    logits = nc.dram_tensor("logits", [P, LCH], F32, kind="ExternalOutput")

    RG = [list(range(N_CORES))]

    with tile.TileContext(nc) as tc:
        with tc.tile_pool(name="const", bufs=1) as cpool, \
             tc.tile_pool(name="msgp", bufs=3) as msgp, \
             tc.tile_pool(name="indp", bufs=4) as indp, \
             tc.tile_pool(name="evac", bufs=3) as evac, \
             tc.tile_pool(name="decp", bufs=2) as decp, \
             tc.tile_pool(name="psA", bufs=2, space="PSUM") as psA, \
             tc.tile_pool(name="psB", bufs=2, space="PSUM") as psB, \
             tc.tile_pool(name="dram", bufs=1, space="DRAM") as dram:

            # constants into SBUF
            xT_s = cpool.tile([P, NPC], F16)
            W1_s = cpool.tile([P, HID_CH], F16)
            W2_s = cpool.tile([P, OUT_CH], F16)
            b1_s = cpool.tile([1, HID_CH], F16)
            b2_s = cpool.tile([1, OUT_CH], F16)
            sq_s = cpool.tile([1, NT * P], F16)
            dk1_s = cpool.tile([P, NT], F32)
            dk2_s = cpool.tile([P, NT], F32)
            ei_s = cpool.tile([P, WC], I16)
            el_s = cpool.tile([P, TOT_CH], F32)
            li_s = cpool.tile([P, WL], I16)
            iota_s = cpool.tile([P, P], F16)
            pcol_s = cpool.tile([P, 1], F32)
            ident_s = cpool.tile([P, P], F16)
            ones_s = cpool.tile([P, 1], F16)
            p_keep = cpool.tile([P, NT, HID_CH], F16)
            q_keep = cpool.tile([P, NT, OUT_CH], F16)
            z_keep = cpool.tile([P, NT, OUT_CH], F16)
            logit_sb = cpool.tile([P, LCH], F32)
            nc.sync.dma_start(out=xT_s[:], in_=xT[:])
            nc.sync.dma_start(out=W1_s[:], in_=W1h[:])
            nc.sync.dma_start(out=W2_s[:], in_=W2h[:])
            nc.sync.dma_start(out=b1_s[:], in_=b1row[:])
            nc.sync.dma_start(out=b2_s[:], in_=b2row[:])
            nc.sync.dma_start(out=sq_s[:], in_=sqrow[:])
            nc.sync.dma_start(out=dk1_s[:], in_=dk1[:])
            nc.sync.dma_start(out=dk2_s[:], in_=dk2[:])
            nc.sync.dma_start(out=ei_s[:], in_=eidx[:])
            nc.sync.dma_start(out=el_s[:], in_=edloc[:])
            nc.sync.dma_start(out=li_s[:], in_=lidx[:])
            nc.vector.memset(ones_s[:], 1.0)
            nc.gpsimd.iota(iota_s[:], pattern=[[1, P]], base=0,
                           channel_multiplier=0,
                           allow_small_or_imprecise_dtypes=True)
            nc.gpsimd.iota(pcol_s[:], pattern=[[0, 1]], base=0,
                           channel_multiplier=1,
                           allow_small_or_imprecise_dtypes=True)
            pmat_s = cpool.tile([P, P], F16)
            nc.gpsimd.iota(pmat_s[:], pattern=[[0, P]], base=0,
                           channel_multiplier=1,
                           allow_small_or_imprecise_dtypes=True)
            nc.vector.tensor_scalar(
                out=ident_s[:], in0=iota_s[:], scalar1=pcol_s[:],
                scalar2=None, op0=mybir.AluOpType.is_equal)

            p_in = dram.tile([NPC, HID_CH], F16)
            PT = dram.tile([N_NODES, HID_CH], F16, addr_space="Shared")
            q_in = dram.tile([NPC, P], F16)
            QT = dram.tile([N_NODES, P], F16, addr_space="Shared")
            z_in = dram.tile([NPC, P], F16)
            ZT = dram.tile([N_NODES, P], F16, addr_space="Shared")

            # ---- stage 1: p~ = (x*dinv) @ W1, per tile; keep + publish
            for t in range(NT):
                m = min(P, NPC - t * P)
                psum_p = psB.tile([P, HID_CH], F32, tag="pp", space="PSUM")
                nc.tensor.matmul(out=psum_p[0:m, :],
                                 lhsT=xT_s[:, t * P:t * P + m],
                                 rhs=W1_s[:], start=True, stop=True)
                nc.scalar.copy(out=p_keep[0:m, t, :], in_=psum_p[0:m, :])
                nc.sync.dma_start(out=p_in[t * P:t * P + m, :],
                                  in_=p_keep[0:m, t, :])

            nc.gpsimd.collective_compute(
                "AllGather", mybir.AluOpType.bypass, replica_groups=RG,
                ins=[p_in.opt()], outs=[PT.opt()])

            def conv_layer(TBL, out_dram, is_conv1):
                for g in meta["groups"]:
                    nch = g["nA"] + g["nB"]
                    msg = msgp.tile([P, NCHG_MAX, P], F16, tag="msg")
                    if g["nA"]:
                        nc.gpsimd.dma_gather(
                            out_ap=msg[:, 0:g["nA"], :],
                            in_ap=TBL[0:A_LIM, :],
                            idxs_ap=ei_s[:, g["colA"]:g["colA"] + g["nA"] * 8],
                            num_idxs=g["nA"] * P, num_idxs_reg=g["nA"] * P,
                            elem_size=P, single_packet=False)
                    if g["nB"]:
                        nc.gpsimd.dma_gather(
                            out_ap=msg[:, g["nA"]:nch, :],
                            in_ap=TBL[B_OFF:N_NODES, :],
                            idxs_ap=ei_s[:, g["colB"]:g["colB"] + g["nB"] * 8],
                            num_idxs=g["nB"] * P, num_idxs_reg=g["nB"] * P,
                            elem_size=P, single_packet=False)
                    for t in g["tiles"]:
                        m = min(P, NPC - t * P)
                        chunks = meta["tile_chunks"][t]
                        if is_conv1:
                            # psum [ch, d], seeded outer(b1, sqrtdeg)
                            ps = psA.tile([HID_CH, P], F32, tag="agg1",
                                          space="PSUM")
                            nc.tensor.matmul(
                                out=ps[:, 0:m], lhsT=b1_s[:],
                                rhs=sq_s[:, t * P:t * P + m],
                                start=True, stop=False)
                        else:
                            # psum [d, ch], seeded outer(sqrtdeg, b2)
                            ps = psA.tile([P, OUT_CH], F32, tag="agg2",
                                          space="PSUM")
                            nc.tensor.matmul(
                                out=ps[0:m, :],
                                lhsT=sq_s[:, t * P:t * P + m],
                                rhs=b2_s[:], start=True, stop=False)
                        for gc in chunks:
                            lc = gc - g["base"]
                            ind = indp.tile([P, P], F16, tag="ind")
                            nc.vector.tensor_scalar(
                                out=ind[:], in0=iota_s[:],
                                scalar1=el_s[:, gc:gc + 1],
                                scalar2=None,
                                op0=mybir.AluOpType.is_equal)
                            if is_conv1:
                                nc.tensor.matmul(
                                    out=ps[:, 0:m], lhsT=msg[:, lc, :],
                                    rhs=ind[:, 0:m],
                                    start=False, stop=False)
                            else:
                                nc.tensor.matmul(
                                    out=ps[0:m, :], lhsT=ind[:, 0:m],
                                    rhs=msg[:, lc, 0:OUT_CH],
                                    start=False, stop=False)
                        # self-loop: += p~[d] (resp. q~[d]) via identity
                        if is_conv1:
                            nc.tensor.matmul(
                                out=ps[:, 0:m], lhsT=p_keep[0:m, t, :],
                                rhs=ident_s[0:m, 0:m],
                                start=False, stop=True)
                            hT = evac.tile([HID_CH, P], F16, tag="hT")
                            nc.scalar.activation(
                                out=hT[:, 0:m], in_=ps[:, 0:m],
                                func=mybir.ActivationFunctionType.Relu)
                            psq = psB.tile([P, HID_CH], F32, tag="pp",
                                           space="PSUM")
                            nc.tensor.matmul(out=psq[0:m, 0:OUT_CH],
                                             lhsT=hT[:, 0:m], rhs=W2_s[:],
                                             start=True, stop=True)
                            qsb = evac.tile([P, P], F16, tag="qev")
                            nc.vector.memset(qsb[:, OUT_CH:P], 0)
                            nc.scalar.activation(
                                out=qsb[0:m, 0:OUT_CH], in_=psq[0:m, 0:OUT_CH],
                                func=mybir.ActivationFunctionType.Copy,
                                scale=dk2_s[0:m, t:t + 1])  # psq slice
                            nc.scalar.copy(out=q_keep[0:m, t, :],
                                           in_=qsb[0:m, 0:OUT_CH])
                            nc.sync.dma_start(
                                out=out_dram[t * P:t * P + m, :],
                                in_=qsb[0:m, :])
                        else:
                            nc.tensor.matmul(
                                out=ps[0:m, :], lhsT=ident_s[0:m, 0:m],
                                rhs=q_keep[0:m, t, :],
                                start=False, stop=True)
                            zsb = evac.tile([P, P], F16, tag="qev")
                            nc.vector.memset(zsb[:, OUT_CH:P], 0)
                            nc.scalar.activation(
                                out=zsb[0:m, 0:OUT_CH], in_=ps[0:m, :],
                                func=mybir.ActivationFunctionType.Copy,
                                scale=dk1_s[0:m, t:t + 1])
                            nc.scalar.copy(out=z_keep[0:m, t, :],
                                           in_=zsb[0:m, 0:OUT_CH])
                            nc.sync.dma_start(
                                out=out_dram[t * P:t * P + m, :],
                                in_=zsb[0:m, :])

            conv_layer(PT, q_in, True)
            nc.gpsimd.collective_compute(
                "AllGather", mybir.AluOpType.bypass, replica_groups=RG,
                ins=[q_in.opt()], outs=[QT.opt()])
            conv_layer(QT, z_in, False)
            nc.gpsimd.collective_compute(
                "AllGather", mybir.AluOpType.bypass, replica_groups=RG,
                ins=[z_in.opt()], outs=[ZT.opt()])

            # ---- decode: za via PE selection from z_keep, zb via
            # transposed dma_gather from ZT; dot = DVE mult + PE reduce
            for b in meta["lbatches"]:
                nch = b["nch"]
                zbT = decp.tile([P, LBATCH * P], F16, tag="zbT")
                av = (0, A_LIM) if b["view"] == 0 else (B_OFF, N_NODES)
                nc.gpsimd.dma_gather(
                    out_ap=zbT[:, 0:nch * P].rearrange(
                        "p (a b) -> p a b", a=1),
                    in_ap=ZT[av[0]:av[1], :],
                    idxs_ap=li_s[:, b["base"] * 8:(b["base"] + nch) * 8],
                    num_idxs=nch * P, num_idxs_reg=nch * P,
                    elem_size=P, single_packet=False, transpose=True)
                # ACT firewall: don't let DVE read dma_gather-written SBUF
                zb2 = decp.tile([P, LBATCH * P], F16, tag="zb2")
                nc.scalar.copy(out=zb2[0:OUT_CH, 0:nch * P],
                               in_=zbT[0:OUT_CH, 0:nch * P])
                la_t = decp.tile([P, LBATCH * P], F16, tag="la")
                nc.sync.dma_start(
                    out=la_t[:, 0:nch * P],
                    in_=laloc[:, b["base"] * P:(b["base"] + nch) * P])
                for ci in range(nch):
                    t = b["tiles"][ci]
                    m = min(P, NPC - t * P)
                    sel = indp.tile([P, P], F16, tag="sel")
                    nc.vector.tensor_tensor(
                        out=sel[:], in0=pmat_s[:],
                        in1=la_t[:, ci * P:ci * P + P],
                        op=mybir.AluOpType.is_equal)
                    psa = psA.tile([HID_CH, P], F32, tag="agg1",
                                   space="PSUM")
                    nc.tensor.matmul(out=psa[0:OUT_CH, :],
                                     lhsT=z_keep[0:m, t, :],
                                     rhs=sel[0:m, :], start=True, stop=True)
                    scr = indp.tile([OUT_CH, P], F16, tag="scr")
                    nc.vector.tensor_tensor(
                        out=scr[:], in0=psa[0:OUT_CH, :],
                        in1=zb2[0:OUT_CH, ci * P:ci * P + P],
                        op=mybir.AluOpType.mult)
                    psl = psB.tile([P, HID_CH], F32, tag="pp",
                                   space="PSUM")
                    nc.tensor.matmul(out=psl[:, 0:1], lhsT=scr[:],
                                     rhs=ones_s[0:OUT_CH, :],
                                     start=True, stop=True)
                    cc = b["base"] + ci
                    nc.scalar.copy(out=logit_sb[:, cc:cc + 1], in_=psl[:, 0:1])
            nc.sync.dma_start(out=logits[:], in_=logit_sb[:])

    nc.compile()
    return nc


_CACHE = {}
TRACE = False          # set True (e.g. from test.py) to capture an NTFF trace
LAST_RESULT = None     # BassKernelResults of the most recent run


def kernel(**inputs):
    meta, cores, perms = _prepare(**inputs)
    key = (meta["TOT_CH"], meta["LCH"], meta["WC"], meta["WL"])
    if key not in _CACHE:
        _CACHE[key] = _build(meta)
    nc = _CACHE[key]
    names = ("xT", "W1h", "W2h", "b1row", "b2row", "sqrow", "dk1", "dk2",
             "eidx", "edloc", "lidx", "laloc")
    in_maps = [{n: c[n] for n in names} for c in cores]
    res = run_bass_kernel_spmd(nc, in_maps, core_ids=list(range(N_CORES)),
                               trace=TRACE)
    global LAST_RESULT
    LAST_RESULT = res
    out = np.empty(N_LABEL, np.float32)
    for k in range(N_CORES):
        vals = res.results[k]["logits"].T.ravel()
        perm = perms[k]
        m = perm >= 0
        out[perm[m]] = vals[m]
    return out


# revision 11
# speedup vs baseline: 1.1499x; 1.0188x over previous
"""GCN link predictor on 8 TRN2 NeuronCores (Bass/Tile) — v2.

Design notes (driven by HW profiling of the v1 baseline, 2.34 ms):
the bottleneck is the GPSIMD (Pool) engine generating SWDGE descriptors for
dma_gather at ~8 ns/index, blocking, with no faster indexed primitive on the
machine (ap_gather ~27 ns/idx, scatter_add wedges the device).  So v2
minimizes gather indices and keeps everything else off the Pool engine:

- Normalization refactor: out[d] = b + dinv[d] * sum_e table[src_e] with
  table rows pre-scaled by dinv[src] (x rows host-scaled; q~ scaled dinv^2
  at evacuation).  Indicators become PURE one-hot -> single-op is_equal on
  DVE, and PSUM is seeded with outer(bias, sqrt(deg)) so the dst-side dinv
  folds into the existing evacuation scale.
- Self-loops never enter the gather path: one identity matmul per dst tile
  adds p~[d] (resp. q~[d]) from SBUF-resident local tiles.
- Decode: label pairs sharded by core(a); the a-side z rows come from PE
  one-hot selection out of SBUF-resident local z tiles (overlaps the last
  AllGather); only the b-side uses dma_gather (transpose=True -> zbT
  [ch, pair]); dot product = DVE multiply + PE ones-reduction.

dma_gather indices are int16, so 40000-row tables are addressed through two
overlapping views: A = rows [0, 32768), B = rows [7232, 40000).
"""

import numpy as np

import concourse.bass as bass
import concourse.bacc as bacc
import concourse.mybir as mybir
import concourse.tile as tile
from concourse.bass_utils import run_bass_kernel_spmd

P = 128
N_NODES = 40000
IN_CH = 128
HID_CH = 128
OUT_CH = 64
N_LABEL = 200000
N_CORES = 8
NPC = N_NODES // N_CORES          # 5000 nodes per core
NT = (NPC + P - 1) // P           # 40 dst tiles per core (last has 8 nodes)
A_LIM = 32768                     # view A = rows [0, 32768)
B_OFF = N_NODES - A_LIM           # 7232; view B = rows [7232, 40000)
GROUP = 4                         # dst tiles per gather group
LBATCH = 32                       # decode chunks per gather batch

F16 = mybir.dt.float16
F32 = mybir.dt.float32
I16 = mybir.dt.int16


def _wrap16(flat):
    """dma_gather / index SBUF image: position n -> [n % 16, n // 16],
    replicated across the 8 groups of 16 partitions. [128, len/16] int16."""
    n = len(flat)
    assert n % 16 == 0
    grid = np.asarray(flat, np.int16).reshape(n // 16, 16).T
    return np.tile(grid, (8, 1))


def _prepare(x, edge_index, edge_label_index, W1, b1, W2, b2):
    src = np.asarray(edge_index[0], np.int64)
    dst = np.asarray(edge_index[1], np.int64)
    deg = (np.bincount(dst, minlength=N_NODES) + 1).astype(np.float64)
    dinv = (1.0 / np.sqrt(deg)).astype(np.float32)
    sqrtdeg = np.sqrt(deg).astype(np.float32)

    # ---- bucket edges by (core, tile), sorted by src within each bucket
    core_of = dst // NPC
    tloc = (dst % NPC) // P
    order = np.lexsort((src, tloc, core_of))
    s_src = src[order]
    s_dst = dst[order]
    key = core_of[order] * NT + tloc[order]
    starts = np.searchsorted(key, np.arange(N_CORES * NT))
    ends = np.searchsorted(key, np.arange(N_CORES * NT) + 1)

    cnt = (ends - starts).reshape(N_CORES, NT)
    fA = np.empty((N_CORES, NT), np.int64)   # forced-A (< B_OFF)
    fB = np.empty((N_CORES, NT), np.int64)   # forced-B (>= A_LIM)
    for k in range(N_CORES):
        for t in range(NT):
            b = k * NT + t
            ss = s_src[starts[b]:ends[b]]
            fA[k, t] = np.searchsorted(ss, B_OFF)
            fB[k, t] = len(ss) - np.searchsorted(ss, A_LIM)
    NCA = np.maximum(1, (fA.max(axis=0) + P - 1) // P)          # [NT]
    nA = np.minimum(cnt - fB, P * NCA[None, :])                 # [cores, NT]
    nA = np.maximum(nA, 0)
    cntB = cnt - nA
    NCB = (cntB.max(axis=0) + P - 1) // P                       # [NT]

    groups = []
    tile_chunks = {}
    gbase = 0
    col = 0
    for g0 in range(0, NT, GROUP):
        ts = list(range(g0, min(g0 + GROUP, NT)))
        gnA = int(NCA[ts].sum())
        gnB = int(NCB[ts].sum())
        groups.append(dict(tiles=ts, base=gbase, nA=gnA, nB=gnB,
                           colA=col, colB=col + gnA * 8))
        ca = gbase
        cb = gbase + gnA
        for t in ts:
            tile_chunks[t] = (list(range(ca, ca + int(NCA[t])))
                              + list(range(cb, cb + int(NCB[t]))))
            ca += int(NCA[t])
            cb += int(NCB[t])
        gbase += gnA + gnB
        col += (gnA + gnB) * 8
    TOT_CH = gbase
    WC = col

    # ---- per-core conv arrays: gather indices + dst-local one-hot columns
    cores = []
    for k in range(N_CORES):
        eidx = np.zeros((P, WC), np.int16)
        edloc = np.full((P, TOT_CH), -1.0, np.float32)  # -1 -> no is_eq match
        for g in groups:
            flatA = []
            flatB = []
            for t in g["tiles"]:
                b = k * NT + t
                ss = s_src[starts[b]:ends[b]]
                dd = s_dst[starts[b]:ends[b]]
                na = int(nA[k, t])
                la = np.full(int(NCA[t]) * P, -1.0, np.float32)
                ia = np.zeros(int(NCA[t]) * P, np.int64)
                ia[:na] = ss[:na]
                la[:na] = (dd[:na] - k * NPC - t * P).astype(np.float32)
                lb_ = np.full(int(NCB[t]) * P, -1.0, np.float32)
                ib = np.zeros(int(NCB[t]) * P, np.int64)
                nb = int(cntB[k, t])
                ib[:nb] = ss[na:na + nb] - B_OFF
                lb_[:nb] = (dd[na:na + nb] - k * NPC - t * P).astype(
                    np.float32)
                flatA.append((ia, la))
                flatB.append((ib, lb_))
            ia = np.concatenate([f[0] for f in flatA])
            ib = np.concatenate([f[0] for f in flatB])
            locs = np.concatenate([f[1] for f in flatA]
                                  + [f[1] for f in flatB])
            if len(ia):
                eidx[:, g["colA"]:g["colA"] + len(ia) // 16] = _wrap16(ia)
            if len(ib):
                eidx[:, g["colB"]:g["colB"] + len(ib) // 16] = _wrap16(ib)
            nch = g["nA"] + g["nB"]
            edloc[:, g["base"]:g["base"] + nch] = locs.reshape(nch, P).T
        cores.append(dict(eidx=eidx, edloc=edloc))

    # ---- decode prep: pairs sharded by core(a), grouped by (a_tile, b_view)
    la_all = np.asarray(edge_label_index[0], np.int64)
    lb_all = np.asarray(edge_label_index[1], np.int64)
    owner = la_all // NPC
    # per (core, a_tile, view) counts to find shared padded chunk counts
    atile = (la_all % NPC) // P
    bview = (lb_all >= A_LIM).astype(np.int64)  # 0 -> A view, 1 -> B view
    cntd = np.zeros((N_CORES, NT, 2), np.int64)
    for k in range(N_CORES):
        m = owner == k
        np.add.at(cntd[k], (atile[m], bview[m]), 1)
    NCD = (cntd.max(axis=0) + P - 1) // P                       # [NT, 2]
    # chunk layout: all view-A chunks (tile-major), then all view-B chunks
    chunksA = []
    chunksB = []
    for t in range(NT):
        for c in range(int(NCD[t, 0])):
            chunksA.append(t)
        for c in range(int(NCD[t, 1])):
            chunksB.append(t)
    LCH = len(chunksA) + len(chunksB)
    lbatches = []
    for v, chs, base in ((0, chunksA, 0), (1, chunksB, len(chunksA))):
        for c0 in range(0, len(chs), LBATCH):
            nch = min(LBATCH, len(chs) - c0)
            lbatches.append(dict(view=v, base=base + c0, nch=nch,
                                 tiles=chs[c0:c0 + nch]))
    WL = LCH * 8

    perms = []
    for k in range(N_CORES):
        m = owner == k
        ga, gb, gidx = la_all[m], lb_all[m], np.nonzero(m)[0]
        at, bv = atile[m], bview[m]
        o = np.lexsort((gb, bv, at))
        ga, gb, gidx, at, bv = ga[o], gb[o], gidx[o], at[o], bv[o]
        lidx = np.zeros((P, WL), np.int16)
        laloc = np.full(LCH * P, -1.0, np.float32)
        perm = np.full(LCH * P, -1, np.int64)
        cbase = {0: 0, 1: len(chunksA)}
        coff = {0: 0, 1: 0}
        for t in range(NT):
            for v in (0, 1):
                mm = (at == t) & (bv == v)
                pa, pb, pi = ga[mm], gb[mm], gidx[mm]
                ncap = int(NCD[t, v]) * P
                assert len(pa) <= ncap
                ids = np.zeros(ncap, np.int64)
                ids[:len(pb)] = pb - (0 if v == 0 else B_OFF)
                start = cbase[v] + coff[v]
                lidx[:, start * 8:(start + int(NCD[t, v])) * 8] = \
                    _wrap16(ids)
                sl = slice(start * P, start * P + len(pa))
                laloc[start * P:(start + int(NCD[t, v])) * P][:len(pa)] = \
                    (pa - k * NPC - t * P).astype(np.float32)
                perm[sl] = pi
                coff[v] += int(NCD[t, v])
        # laloc broadcast image: [128 partitions, LCH*128] fp16, value =
        # a_loc of the pair in that column (same in every partition)
        lab = np.broadcast_to(laloc[None, :], (P, LCH * P)).astype(np.float16)
        cores[k]["lidx"] = lidx
        cores[k]["laloc"] = np.ascontiguousarray(lab)
        perms.append(perm)

    # ---- dense inputs per core
    x = np.asarray(x, np.float32)
    for k in range(N_CORES):
        xk = x[k * NPC:(k + 1) * NPC] * dinv[k * NPC:(k + 1) * NPC, None]
        cores[k]["xT"] = np.ascontiguousarray(xk.T).astype(np.float16)
        cores[k]["W1h"] = np.asarray(W1, np.float32).astype(np.float16)
        cores[k]["W2h"] = np.asarray(W2, np.float32).astype(np.float16)
        cores[k]["b1row"] = np.asarray(b1, np.float32).astype(
            np.float16).reshape(1, HID_CH)
        cores[k]["b2row"] = np.asarray(b2, np.float32).astype(
            np.float16).reshape(1, OUT_CH)
        sq = np.zeros((1, NT * P), np.float16)
        sq[0, :NPC] = sqrtdeg[k * NPC:(k + 1) * NPC]
        cores[k]["sqrow"] = sq
        dk1 = np.ones((P, NT), np.float32)
        dk2 = np.ones((P, NT), np.float32)
        dv = dinv[k * NPC:(k + 1) * NPC]
        for t in range(NT):
            m = min(P, NPC - t * P)
            dk1[:m, t] = dv[t * P:t * P + m]
            dk2[:m, t] = dv[t * P:t * P + m] ** 2
        cores[k]["dk1"] = dk1
        cores[k]["dk2"] = dk2

    meta = dict(groups=groups, tile_chunks=tile_chunks, TOT_CH=TOT_CH,
                WC=WC, lbatches=lbatches, LCH=LCH, WL=WL,
                NCD=[[int(v) for v in row] for row in NCD])
    return meta, cores, perms


def _build(meta):
    TOT_CH, WC, LCH, WL = (meta["TOT_CH"], meta["WC"],
                           meta["LCH"], meta["WL"])
    NCHG_MAX = max(g["nA"] + g["nB"] for g in meta["groups"])

    nc = bacc.Bacc("TRN2", target_bir_lowering=False, debug=False,
                   num_devices=N_CORES)
    xT = nc.dram_tensor("xT", [P, NPC], F16, kind="ExternalInput")
    W1h = nc.dram_tensor("W1h", [P, HID_CH], F16, kind="ExternalInput")
    W2h = nc.dram_tensor("W2h", [P, OUT_CH], F16, kind="ExternalInput")
    b1row = nc.dram_tensor("b1row", [1, HID_CH], F16, kind="ExternalInput")
    b2row = nc.dram_tensor("b2row", [1, OUT_CH], F16, kind="ExternalInput")
    sqrow = nc.dram_tensor("sqrow", [1, NT * P], F16, kind="ExternalInput")
    dk1 = nc.dram_tensor("dk1", [P, NT], F32, kind="ExternalInput")
    dk2 = nc.dram_tensor("dk2", [P, NT], F32, kind="ExternalInput")
    eidx = nc.dram_tensor("eidx", [P, WC], I16, kind="ExternalInput")
    edloc = nc.dram_tensor("edloc", [P, TOT_CH], F32, kind="ExternalInput")
    lidx = nc.dram_tensor("lidx", [P, WL], I16, kind="ExternalInput")
    laloc = nc.dram_tensor("laloc", [P, LCH * P], F16, kind="ExternalInput")
    logits = nc.dram_tensor("logits", [P, LCH], F32, kind="ExternalOutput")

    RG = [list(range(N_CORES))]

    with tile.TileContext(nc) as tc:
        with tc.tile_pool(name="const", bufs=1) as cpool, \
             tc.tile_pool(name="msgp", bufs=4) as msgp, \
             tc.tile_pool(name="indp", bufs=4) as indp, \
             tc.tile_pool(name="evac", bufs=3) as evac, \
             tc.tile_pool(name="decp", bufs=2) as decp, \
             tc.tile_pool(name="psA", bufs=2, space="PSUM") as psA, \
             tc.tile_pool(name="psB", bufs=2, space="PSUM") as psB, \
             tc.tile_pool(name="dram", bufs=1, space="DRAM") as dram:

            # constants into SBUF
            xT_s = cpool.tile([P, NPC], F16)
            W1_s = cpool.tile([P, HID_CH], F16)
            W2_s = cpool.tile([P, OUT_CH], F16)
            b1_s = cpool.tile([1, HID_CH], F16)
            b2_s = cpool.tile([1, OUT_CH], F16)
            sq_s = cpool.tile([1, NT * P], F16)
            dk1_s = cpool.tile([P, NT], F32)
            dk2_s = cpool.tile([P, NT], F32)
            ei_s = cpool.tile([P, WC], I16)
            el_s = cpool.tile([P, TOT_CH], F32)
            li_s = cpool.tile([P, WL], I16)
            iota_s = cpool.tile([P, P], F16)
            pcol_s = cpool.tile([P, 1], F32)
            ident_s = cpool.tile([P, P], F16)
            ones_s = cpool.tile([P, 1], F16)
            p_keep = cpool.tile([P, NT, HID_CH], F16)
            q_keep = cpool.tile([P, NT, OUT_CH], F16)
            z_keep = cpool.tile([P, NT, OUT_CH], F16)
            logit_sb = cpool.tile([P, LCH], F32)
            nc.sync.dma_start(out=xT_s[:], in_=xT[:])
            nc.sync.dma_start(out=W1_s[:], in_=W1h[:])
            nc.sync.dma_start(out=W2_s[:], in_=W2h[:])
            nc.sync.dma_start(out=b1_s[:], in_=b1row[:])
            nc.sync.dma_start(out=b2_s[:], in_=b2row[:])
            nc.sync.dma_start(out=sq_s[:], in_=sqrow[:])
            nc.sync.dma_start(out=dk1_s[:], in_=dk1[:])
            nc.sync.dma_start(out=dk2_s[:], in_=dk2[:])
            nc.sync.dma_start(out=ei_s[:], in_=eidx[:])
            nc.sync.dma_start(out=el_s[:], in_=edloc[:])
            nc.sync.dma_start(out=li_s[:], in_=lidx[:])
            nc.vector.memset(ones_s[:], 1.0)
            nc.gpsimd.iota(iota_s[:], pattern=[[1, P]], base=0,
                           channel_multiplier=0,
                           allow_small_or_imprecise_dtypes=True)
            nc.gpsimd.iota(pcol_s[:], pattern=[[0, 1]], base=0,
                           channel_multiplier=1,
                           allow_small_or_imprecise_dtypes=True)
            pmat_s = cpool.tile([P, P], F16)
            nc.gpsimd.iota(pmat_s[:], pattern=[[0, P]], base=0,
                           channel_multiplier=1,
                           allow_small_or_imprecise_dtypes=True)
            nc.vector.tensor_scalar(
                out=ident_s[:], in0=iota_s[:], scalar1=pcol_s[:],
                scalar2=None, op0=mybir.AluOpType.is_equal)

            p_in = dram.tile([NPC, HID_CH], F16)
            PT = dram.tile([N_NODES, HID_CH], F16, addr_space="Shared")
            q_in = dram.tile([NPC, P], F16)
            QT = dram.tile([N_NODES, P], F16, addr_space="Shared")
            z_in = dram.tile([NPC, P], F16)
            ZT = dram.tile([N_NODES, P], F16, addr_space="Shared")

            # ---- stage 1: p~ = (x*dinv) @ W1, per tile; keep + publish
            for t in range(NT):
                m = min(P, NPC - t * P)
                psum_p = psB.tile([P, HID_CH], F32, tag="pp", space="PSUM")
                nc.tensor.matmul(out=psum_p[0:m, :],
                                 lhsT=xT_s[:, t * P:t * P + m],
                                 rhs=W1_s[:], start=True, stop=True)
                nc.scalar.copy(out=p_keep[0:m, t, :], in_=psum_p[0:m, :])
                nc.sync.dma_start(out=p_in[t * P:t * P + m, :],
                                  in_=p_keep[0:m, t, :])

            nc.gpsimd.collective_compute(
                "AllGather", mybir.AluOpType.bypass, replica_groups=RG,
                ins=[p_in.opt()], outs=[PT.opt()])

            def conv_layer(TBL, out_dram, is_conv1):
                for g in meta["groups"]:
                    nch = g["nA"] + g["nB"]
                    msg = msgp.tile([P, NCHG_MAX, P], F16, tag="msg")
                    if g["nA"]:
                        nc.gpsimd.dma_gather(
                            out_ap=msg[:, 0:g["nA"], :],
                            in_ap=TBL[0:A_LIM, :],
                            idxs_ap=ei_s[:, g["colA"]:g["colA"] + g["nA"] * 8],
                            num_idxs=g["nA"] * P, num_idxs_reg=g["nA"] * P,
                            elem_size=P, single_packet=False)
                    if g["nB"]:
                        nc.gpsimd.dma_gather(
                            out_ap=msg[:, g["nA"]:nch, :],
                            in_ap=TBL[B_OFF:N_NODES, :],
                            idxs_ap=ei_s[:, g["colB"]:g["colB"] + g["nB"] * 8],
                            num_idxs=g["nB"] * P, num_idxs_reg=g["nB"] * P,
                            elem_size=P, single_packet=False)
                    for t in g["tiles"]:
                        m = min(P, NPC - t * P)
                        chunks = meta["tile_chunks"][t]
                        if is_conv1:
                            # psum [ch, d], seeded outer(b1, sqrtdeg)
                            ps = psA.tile([HID_CH, P], F32, tag="agg1",
                                          space="PSUM")
                            nc.tensor.matmul(
                                out=ps[:, 0:m], lhsT=b1_s[:],
                                rhs=sq_s[:, t * P:t * P + m],
                                start=True, stop=False)
                        else:
                            # psum [d, ch], seeded outer(sqrtdeg, b2)
                            ps = psA.tile([P, OUT_CH], F32, tag="agg2",
                                          space="PSUM")
                            nc.tensor.matmul(
                                out=ps[0:m, :],
                                lhsT=sq_s[:, t * P:t * P + m],
                                rhs=b2_s[:], start=True, stop=False)
                        for gc in chunks:
                            lc = gc - g["base"]
                            ind = indp.tile([P, P], F16, tag="ind")
                            nc.vector.tensor_scalar(
                                out=ind[:], in0=iota_s[:],
                                scalar1=el_s[:, gc:gc + 1],
                                scalar2=None,
                                op0=mybir.AluOpType.is_equal)
                            if is_conv1:
                                nc.tensor.matmul(
                                    out=ps[:, 0:m], lhsT=msg[:, lc, :],
                                    rhs=ind[:, 0:m],
                                    start=False, stop=False)
                            else:
                                nc.tensor.matmul(
                                    out=ps[0:m, :], lhsT=ind[:, 0:m],
                                    rhs=msg[:, lc, 0:OUT_CH],
                                    start=False, stop=False)
                        # self-loop: += p~[d] (resp. q~[d]) via identity
                        if is_conv1:
                            nc.tensor.matmul(
                                out=ps[:, 0:m], lhsT=p_keep[0:m, t, :],
                                rhs=ident_s[0:m, 0:m],
                                start=False, stop=True)
                            hT = evac.tile([HID_CH, P], F16, tag="hT")
                            nc.scalar.activation(
                                out=hT[:, 0:m], in_=ps[:, 0:m],
                                func=mybir.ActivationFunctionType.Relu)
                            psq = psB.tile([P, HID_CH], F32, tag="pp",
                                           space="PSUM")
                            nc.tensor.matmul(out=psq[0:m, 0:OUT_CH],
                                             lhsT=hT[:, 0:m], rhs=W2_s[:],
                                             start=True, stop=True)
                            qsb = evac.tile([P, P], F16, tag="qev")
                            nc.vector.memset(qsb[:, OUT_CH:P], 0)
                            nc.scalar.activation(
                                out=qsb[0:m, 0:OUT_CH], in_=psq[0:m, 0:OUT_CH],
                                func=mybir.ActivationFunctionType.Copy,
                                scale=dk2_s[0:m, t:t + 1])  # psq slice
                            nc.scalar.copy(out=q_keep[0:m, t, :],
                                           in_=qsb[0:m, 0:OUT_CH])
                            nc.sync.dma_start(
                                out=out_dram[t * P:t * P + m, :],
                                in_=qsb[0:m, :])
                        else:
                            nc.tensor.matmul(
                                out=ps[0:m, :], lhsT=ident_s[0:m, 0:m],
                                rhs=q_keep[0:m, t, :],
                                start=False, stop=True)
                            zsb = evac.tile([P, P], F16, tag="qev")
                            nc.vector.memset(zsb[:, OUT_CH:P], 0)
                            nc.scalar.activation(
                                out=zsb[0:m, 0:OUT_CH], in_=ps[0:m, :],
                                func=mybir.ActivationFunctionType.Copy,
                                scale=dk1_s[0:m, t:t + 1])
                            nc.scalar.copy(out=z_keep[0:m, t, :],
                                           in_=zsb[0:m, 0:OUT_CH])
                            nc.sync.dma_start(
                                out=out_dram[t * P:t * P + m, :],
                                in_=zsb[0:m, :])

            conv_layer(PT, q_in, True)
            nc.gpsimd.collective_compute(
                "AllGather", mybir.AluOpType.bypass, replica_groups=RG,
                ins=[q_in.opt()], outs=[QT.opt()])
            conv_layer(QT, z_in, False)
            nc.gpsimd.collective_compute(
                "AllGather", mybir.AluOpType.bypass, replica_groups=RG,
                ins=[z_in.opt()], outs=[ZT.opt()])

            # ---- decode: za via PE selection from z_keep, zb via
            # transposed dma_gather from ZT; dot = DVE mult + PE reduce
            for b in meta["lbatches"]:
                nch = b["nch"]
                zbT = decp.tile([P, LBATCH * P], F16, tag="zbT")
                av = (0, A_LIM) if b["view"] == 0 else (B_OFF, N_NODES)
                nc.gpsimd.dma_gather(
                    out_ap=zbT[:, 0:nch * P].rearrange(
                        "p (a b) -> p a b", a=1),
                    in_ap=ZT[av[0]:av[1], :],
                    idxs_ap=li_s[:, b["base"] * 8:(b["base"] + nch) * 8],
                    num_idxs=nch * P, num_idxs_reg=nch * P,
                    elem_size=P, single_packet=False, transpose=True)
                # ACT firewall: don't let DVE read dma_gather-written SBUF
                zb2 = decp.tile([P, LBATCH * P], F16, tag="zb2")
                nc.scalar.copy(out=zb2[0:OUT_CH, 0:nch * P],
                               in_=zbT[0:OUT_CH, 0:nch * P])
                la_t = decp.tile([P, LBATCH * P], F16, tag="la")
                nc.sync.dma_start(
                    out=la_t[:, 0:nch * P],
                    in_=laloc[:, b["base"] * P:(b["base"] + nch) * P])
                for ci in range(nch):
                    t = b["tiles"][ci]
                    m = min(P, NPC - t * P)
                    sel = indp.tile([P, P], F16, tag="sel")
                    nc.vector.tensor_tensor(
                        out=sel[:], in0=pmat_s[:],
                        in1=la_t[:, ci * P:ci * P + P],
                        op=mybir.AluOpType.is_equal)
                    psa = psA.tile([HID_CH, P], F32, tag="agg1",
                                   space="PSUM")
                    nc.tensor.matmul(out=psa[0:OUT_CH, :],
                                     lhsT=z_keep[0:m, t, :],
                                     rhs=sel[0:m, :], start=True, stop=True)
                    scr = indp.tile([OUT_CH, P], F16, tag="scr")
                    nc.vector.tensor_tensor(
                        out=scr[:], in0=psa[0:OUT_CH, :],
                        in1=zb2[0:OUT_CH, ci * P:ci * P + P],
                        op=mybir.AluOpType.mult)
                    psl = psB.tile([P, HID_CH], F32, tag="pp",
                                   space="PSUM")
                    nc.tensor.matmul(out=psl[:, 0:1], lhsT=scr[:],
                                     rhs=ones_s[0:OUT_CH, :],
                                     start=True, stop=True)
                    cc = b["base"] + ci
                    nc.scalar.copy(out=logit_sb[:, cc:cc + 1], in_=psl[:, 0:1])
            nc.sync.dma_start(out=logits[:], in_=logit_sb[:])

    nc.compile()
    return nc


_CACHE = {}
TRACE = False          # set True (e.g. from test.py) to capture an NTFF trace
LAST_RESULT = None     # BassKernelResults of the most recent run


def kernel(**inputs):
    meta, cores, perms = _prepare(**inputs)
    key = (meta["TOT_CH"], meta["LCH"], meta["WC"], meta["WL"])
    if key not in _CACHE:
        _CACHE[key] = _build(meta)
    nc = _CACHE[key]
    names = ("xT", "W1h", "W2h", "b1row", "b2row", "sqrow", "dk1", "dk2",
             "eidx", "edloc", "lidx", "laloc")
    in_maps = [{n: c[n] for n in names} for c in cores]
    res = run_bass_kernel_spmd(nc, in_maps, core_ids=list(range(N_CORES)),
                               trace=TRACE)
    global LAST_RESULT
    LAST_RESULT = res
    out = np.empty(N_LABEL, np.float32)
    for k in range(N_CORES):
        vals = res.results[k]["logits"].T.ravel()
        perm = perms[k]
        m = perm >= 0
        out[perm[m]] = vals[m]
    return out


# revision 15
# speedup vs baseline: 1.1657x; 1.0138x over previous
"""GCN link predictor on 8 TRN2 NeuronCores (Bass/Tile) — v2.

Design notes (driven by HW profiling of the v1 baseline, 2.34 ms):
the bottleneck is the GPSIMD (Pool) engine generating SWDGE descriptors for
dma_gather at ~8 ns/index, blocking, with no faster indexed primitive on the
machine (ap_gather ~27 ns/idx, scatter_add wedges the device).  So v2
minimizes gather indices and keeps everything else off the Pool engine:

- Normalization refactor: out[d] = b + dinv[d] * sum_e table[src_e] with
  table rows pre-scaled by dinv[src] (x rows host-scaled; q~ scaled dinv^2
  at evacuation).  Indicators become PURE one-hot -> single-op is_equal on
  DVE, and PSUM is seeded with outer(bias, sqrt(deg)) so the dst-side dinv
  folds into the existing evacuation scale.
- Self-loops never enter the gather path: one identity matmul per dst tile
  adds p~[d] (resp. q~[d]) from SBUF-resident local tiles.
- Decode: label pairs sharded by core(a); the a-side z rows come from PE
  one-hot selection out of SBUF-resident local z tiles (overlaps the last
  AllGather); only the b-side uses dma_gather (transpose=True -> zbT
  [ch, pair]); dot product = DVE multiply + PE ones-reduction.

dma_gather indices are int16, so 40000-row tables are addressed through two
overlapping views: A = rows [0, 32768), B = rows [7232, 40000).
"""

import numpy as np

import concourse.bass as bass
import concourse.bacc as bacc
import concourse.mybir as mybir
import concourse.tile as tile
from concourse.bass_utils import run_bass_kernel_spmd

P = 128
N_NODES = 40000
IN_CH = 128
HID_CH = 128
OUT_CH = 64
N_LABEL = 200000
N_CORES = 8
NPC = N_NODES // N_CORES          # 5000 nodes per core
NT = (NPC + P - 1) // P           # 40 dst tiles per core (last has 8 nodes)
A_LIM = 32768                     # view A = rows [0, 32768)
B_OFF = N_NODES - A_LIM           # 7232; view B = rows [7232, 40000)
GROUP = 4                         # dst tiles per gather group
LBATCH = 32                       # decode chunks per gather batch

F16 = mybir.dt.float16
F32 = mybir.dt.float32
I16 = mybir.dt.int16


def _wrap16(flat):
    """dma_gather / index SBUF image: position n -> [n % 16, n // 16],
    replicated across the 8 groups of 16 partitions. [128, len/16] int16."""
    n = len(flat)
    assert n % 16 == 0
    grid = np.asarray(flat, np.int16).reshape(n // 16, 16).T
    return np.tile(grid, (8, 1))


def _prepare(x, edge_index, edge_label_index, W1, b1, W2, b2):
    src = np.asarray(edge_index[0], np.int64)
    dst = np.asarray(edge_index[1], np.int64)
    deg = (np.bincount(dst, minlength=N_NODES) + 1).astype(np.float64)
    dinv = (1.0 / np.sqrt(deg)).astype(np.float32)
    sqrtdeg = np.sqrt(deg).astype(np.float32)

    # ---- bucket edges by (core, tile), sorted by src within each bucket
    core_of = dst // NPC
    tloc = (dst % NPC) // P
    order = np.lexsort((src, tloc, core_of))
    s_src = src[order]
    s_dst = dst[order]
    key = core_of[order] * NT + tloc[order]
    starts = np.searchsorted(key, np.arange(N_CORES * NT))
    ends = np.searchsorted(key, np.arange(N_CORES * NT) + 1)

    cnt = (ends - starts).reshape(N_CORES, NT)
    fA = np.empty((N_CORES, NT), np.int64)   # forced-A (< B_OFF)
    fB = np.empty((N_CORES, NT), np.int64)   # forced-B (>= A_LIM)
    for k in range(N_CORES):
        for t in range(NT):
            b = k * NT + t
            ss = s_src[starts[b]:ends[b]]
            fA[k, t] = np.searchsorted(ss, B_OFF)
            fB[k, t] = len(ss) - np.searchsorted(ss, A_LIM)
    NCA = np.maximum(1, (fA.max(axis=0) + P - 1) // P)          # [NT]
    nA = np.minimum(cnt - fB, P * NCA[None, :])                 # [cores, NT]
    nA = np.maximum(nA, 0)
    cntB = cnt - nA
    NCB = (cntB.max(axis=0) + P - 1) // P                       # [NT]

    groups = []
    tile_chunks = {}
    gbase = 0
    col = 0
    for g0 in range(0, NT, GROUP):
        ts = list(range(g0, min(g0 + GROUP, NT)))
        gnA = int(NCA[ts].sum())
        gnB = int(NCB[ts].sum())
        groups.append(dict(tiles=ts, base=gbase, nA=gnA, nB=gnB,
                           colA=col, colB=col + gnA * 8))
        ca = gbase
        cb = gbase + gnA
        for t in ts:
            tile_chunks[t] = (list(range(ca, ca + int(NCA[t])))
                              + list(range(cb, cb + int(NCB[t]))))
            ca += int(NCA[t])
            cb += int(NCB[t])
        gbase += gnA + gnB
        col += (gnA + gnB) * 8
    TOT_CH = gbase
    WC = col

    # ---- per-core conv arrays: gather indices + dst-local one-hot columns
    cores = []
    for k in range(N_CORES):
        eidx = np.zeros((P, WC), np.int16)
        edloc = np.full((P, TOT_CH), -1.0, np.float32)  # -1 -> no is_eq match
        for g in groups:
            flatA = []
            flatB = []
            for t in g["tiles"]:
                b = k * NT + t
                ss = s_src[starts[b]:ends[b]]
                dd = s_dst[starts[b]:ends[b]]
                na = int(nA[k, t])
                la = np.full(int(NCA[t]) * P, -1.0, np.float32)
                ia = np.zeros(int(NCA[t]) * P, np.int64)
                ia[:na] = ss[:na]
                la[:na] = (dd[:na] - k * NPC - t * P).astype(np.float32)
                lb_ = np.full(int(NCB[t]) * P, -1.0, np.float32)
                ib = np.zeros(int(NCB[t]) * P, np.int64)
                nb = int(cntB[k, t])
                ib[:nb] = ss[na:na + nb] - B_OFF
                lb_[:nb] = (dd[na:na + nb] - k * NPC - t * P).astype(
                    np.float32)
                flatA.append((ia, la))
                flatB.append((ib, lb_))
            ia = np.concatenate([f[0] for f in flatA])
            ib = np.concatenate([f[0] for f in flatB])
            locs = np.concatenate([f[1] for f in flatA]
                                  + [f[1] for f in flatB])
            if len(ia):
                eidx[:, g["colA"]:g["colA"] + len(ia) // 16] = _wrap16(ia)
            if len(ib):
                eidx[:, g["colB"]:g["colB"] + len(ib) // 16] = _wrap16(ib)
            nch = g["nA"] + g["nB"]
            edloc[:, g["base"]:g["base"] + nch] = locs.reshape(nch, P).T
        cores.append(dict(eidx=eidx, edloc=edloc))

    # ---- decode prep: pairs sharded by core(a), grouped by (a_tile, b_view)
    la_all = np.asarray(edge_label_index[0], np.int64)
    lb_all = np.asarray(edge_label_index[1], np.int64)
    owner = la_all // NPC
    # per (core, a_tile, view) counts to find shared padded chunk counts
    atile = (la_all % NPC) // P
    bview = (lb_all >= A_LIM).astype(np.int64)  # 0 -> A view, 1 -> B view
    cntd = np.zeros((N_CORES, NT, 2), np.int64)
    for k in range(N_CORES):
        m = owner == k
        np.add.at(cntd[k], (atile[m], bview[m]), 1)
    NCD = (cntd.max(axis=0) + P - 1) // P                       # [NT, 2]
    # chunk layout: all view-A chunks (tile-major), then all view-B chunks
    chunksA = []
    chunksB = []
    for t in range(NT):
        for c in range(int(NCD[t, 0])):
            chunksA.append(t)
        for c in range(int(NCD[t, 1])):
            chunksB.append(t)
    LCH = len(chunksA) + len(chunksB)
    lbatches = []
    for v, chs, base in ((0, chunksA, 0), (1, chunksB, len(chunksA))):
        for c0 in range(0, len(chs), LBATCH):
            nch = min(LBATCH, len(chs) - c0)
            lbatches.append(dict(view=v, base=base + c0, nch=nch,
                                 tiles=chs[c0:c0 + nch]))
    WL = LCH * 8

    perms = []
    for k in range(N_CORES):
        m = owner == k
        ga, gb, gidx = la_all[m], lb_all[m], np.nonzero(m)[0]
        at, bv = atile[m], bview[m]
        o = np.lexsort((gb, bv, at))
        ga, gb, gidx, at, bv = ga[o], gb[o], gidx[o], at[o], bv[o]
        lidx = np.zeros((P, WL), np.int16)
        laloc = np.full(LCH * P, -1.0, np.float32)
        perm = np.full(LCH * P, -1, np.int64)
        cbase = {0: 0, 1: len(chunksA)}
        coff = {0: 0, 1: 0}
        for t in range(NT):
            for v in (0, 1):
                mm = (at == t) & (bv == v)
                pa, pb, pi = ga[mm], gb[mm], gidx[mm]
                ncap = int(NCD[t, v]) * P
                assert len(pa) <= ncap
                ids = np.zeros(ncap, np.int64)
                ids[:len(pb)] = pb - (0 if v == 0 else B_OFF)
                start = cbase[v] + coff[v]
                lidx[:, start * 8:(start + int(NCD[t, v])) * 8] = \
                    _wrap16(ids)
                sl = slice(start * P, start * P + len(pa))
                laloc[start * P:(start + int(NCD[t, v])) * P][:len(pa)] = \
                    (pa - k * NPC - t * P).astype(np.float32)
                perm[sl] = pi
                coff[v] += int(NCD[t, v])
        # laloc broadcast image: [128 partitions, LCH*128] fp16, value =
        # a_loc of the pair in that column (same in every partition)
        lab = np.broadcast_to(laloc[None, :], (P, LCH * P)).astype(np.float16)
        cores[k]["lidx"] = lidx
        cores[k]["laloc"] = np.ascontiguousarray(lab)
        perms.append(perm)

    # ---- dense inputs per core
    x = np.asarray(x, np.float32)
    for k in range(N_CORES):
        xk = x[k * NPC:(k + 1) * NPC] * dinv[k * NPC:(k + 1) * NPC, None]
        cores[k]["xT"] = np.ascontiguousarray(xk.T).astype(np.float16)
        cores[k]["W1h"] = np.asarray(W1, np.float32).astype(np.float16)
        cores[k]["W2h"] = np.asarray(W2, np.float32).astype(np.float16)
        cores[k]["b1row"] = np.asarray(b1, np.float32).astype(
            np.float16).reshape(1, HID_CH)
        cores[k]["b2row"] = np.asarray(b2, np.float32).astype(
            np.float16).reshape(1, OUT_CH)
        sq = np.zeros((1, NT * P), np.float16)
        sq[0, :NPC] = sqrtdeg[k * NPC:(k + 1) * NPC]
        cores[k]["sqrow"] = sq
        dk1 = np.ones((P, NT), np.float32)
        dk2 = np.ones((P, NT), np.float32)
        dv = dinv[k * NPC:(k + 1) * NPC]
        for t in range(NT):
            m = min(P, NPC - t * P)
            dk1[:m, t] = dv[t * P:t * P + m]
            dk2[:m, t] = dv[t * P:t * P + m] ** 2
        cores[k]["dk1"] = dk1
        cores[k]["dk2"] = dk2

    meta = dict(groups=groups, tile_chunks=tile_chunks, TOT_CH=TOT_CH,
                WC=WC, lbatches=lbatches, LCH=LCH, WL=WL,
                NCD=[[int(v) for v in row] for row in NCD])
    return meta, cores, perms


def _build(meta):
    TOT_CH, WC, LCH, WL = (meta["TOT_CH"], meta["WC"],
                           meta["LCH"], meta["WL"])
    NCHG_MAX = max(g["nA"] + g["nB"] for g in meta["groups"])

    nc = bacc.Bacc("TRN2", target_bir_lowering=False, debug=False,
                   num_devices=N_CORES)
    xT = nc.dram_tensor("xT", [P, NPC], F16, kind="ExternalInput")
    W1h = nc.dram_tensor("W1h", [P, HID_CH], F16, kind="ExternalInput")
    W2h = nc.dram_tensor("W2h", [P, OUT_CH], F16, kind="ExternalInput")
    b1row = nc.dram_tensor("b1row", [1, HID_CH], F16, kind="ExternalInput")
    b2row = nc.dram_tensor("b2row", [1, OUT_CH], F16, kind="ExternalInput")
    sqrow = nc.dram_tensor("sqrow", [1, NT * P], F16, kind="ExternalInput")
    dk1 = nc.dram_tensor("dk1", [P, NT], F32, kind="ExternalInput")
    dk2 = nc.dram_tensor("dk2", [P, NT], F32, kind="ExternalInput")
    eidx = nc.dram_tensor("eidx", [P, WC], I16, kind="ExternalInput")
    edloc = nc.dram_tensor("edloc", [P, TOT_CH], F32, kind="ExternalInput")
    lidx = nc.dram_tensor("lidx", [P, WL], I16, kind="ExternalInput")
    laloc = nc.dram_tensor("laloc", [P, LCH * P], F16, kind="ExternalInput")
    logits = nc.dram_tensor("logits", [P, LCH], F32, kind="ExternalOutput")

    RG = [list(range(N_CORES))]

    with tile.TileContext(nc) as tc:
        with tc.tile_pool(name="const", bufs=1) as cpool, \
             tc.tile_pool(name="msgp", bufs=4) as msgp, \
             tc.tile_pool(name="indp", bufs=4) as indp, \
             tc.tile_pool(name="evac", bufs=3) as evac, \
             tc.tile_pool(name="decp", bufs=2) as decp, \
             tc.tile_pool(name="psA", bufs=2, space="PSUM") as psA, \
             tc.tile_pool(name="psB", bufs=2, space="PSUM") as psB, \
             tc.tile_pool(name="dram", bufs=1, space="DRAM") as dram:

            # constants into SBUF
            xT_s = cpool.tile([P, NPC], F16)
            W1_s = cpool.tile([P, HID_CH], F16)
            W2_s = cpool.tile([P, OUT_CH], F16)
            b1_s = cpool.tile([1, HID_CH], F16)
            b2_s = cpool.tile([1, OUT_CH], F16)
            sq_s = cpool.tile([1, NT * P], F16)
            dk1_s = cpool.tile([P, NT], F32)
            dk2_s = cpool.tile([P, NT], F32)
            ei_s = cpool.tile([P, WC], I16)
            el_s = cpool.tile([P, TOT_CH], F32)
            li_s = cpool.tile([P, WL], I16)
            iota_s = cpool.tile([P, P], F16)
            pcol_s = cpool.tile([P, 1], F32)
            ident_s = cpool.tile([P, P], F16)
            ones_s = cpool.tile([P, 1], F16)
            p_keep = cpool.tile([P, NT, HID_CH], F16)
            q_keep = cpool.tile([P, NT, OUT_CH], F16)
            z_keep = cpool.tile([P, NT, OUT_CH], F16)
            logit_sb = cpool.tile([P, LCH], F32)
            nc.sync.dma_start(out=xT_s[:], in_=xT[:])
            nc.sync.dma_start(out=W1_s[:], in_=W1h[:])
            nc.sync.dma_start(out=W2_s[:], in_=W2h[:])
            nc.sync.dma_start(out=b1_s[:], in_=b1row[:])
            nc.sync.dma_start(out=b2_s[:], in_=b2row[:])
            nc.sync.dma_start(out=sq_s[:], in_=sqrow[:])
            nc.sync.dma_start(out=dk1_s[:], in_=dk1[:])
            nc.sync.dma_start(out=dk2_s[:], in_=dk2[:])
            nc.sync.dma_start(out=ei_s[:], in_=eidx[:])
            nc.sync.dma_start(out=el_s[:], in_=edloc[:])
            nc.sync.dma_start(out=li_s[:], in_=lidx[:])
            nc.vector.memset(ones_s[:], 1.0)
            nc.gpsimd.iota(iota_s[:], pattern=[[1, P]], base=0,
                           channel_multiplier=0,
                           allow_small_or_imprecise_dtypes=True)
            nc.gpsimd.iota(pcol_s[:], pattern=[[0, 1]], base=0,
                           channel_multiplier=1,
                           allow_small_or_imprecise_dtypes=True)
            pmat_s = cpool.tile([P, P], F16)
            nc.gpsimd.iota(pmat_s[:], pattern=[[0, P]], base=0,
                           channel_multiplier=1,
                           allow_small_or_imprecise_dtypes=True)
            nc.vector.tensor_scalar(
                out=ident_s[:], in0=iota_s[:], scalar1=pcol_s[:],
                scalar2=None, op0=mybir.AluOpType.is_equal)

            p_in = dram.tile([NPC, HID_CH], F16)
            PT = dram.tile([N_NODES, HID_CH], F16, addr_space="Shared")
            q_in = dram.tile([NPC, P], F16)
            QT = dram.tile([N_NODES, P], F16, addr_space="Shared")
            z_in = dram.tile([NPC, P], F16)
            ZT = dram.tile([N_NODES, P], F16, addr_space="Shared")

            # ---- stage 1: p~ = (x*dinv) @ W1, per tile; keep + publish
            for t in range(NT):
                m = min(P, NPC - t * P)
                psum_p = psB.tile([P, HID_CH], F32, tag="pp", space="PSUM")
                nc.tensor.matmul(out=psum_p[0:m, :],
                                 lhsT=xT_s[:, t * P:t * P + m],
                                 rhs=W1_s[:], start=True, stop=True)
                nc.scalar.copy(out=p_keep[0:m, t, :], in_=psum_p[0:m, :])
                nc.sync.dma_start(out=p_in[t * P:t * P + m, :],
                                  in_=p_keep[0:m, t, :])

            nc.gpsimd.collective_compute(
                "AllGather", mybir.AluOpType.bypass, replica_groups=RG,
                ins=[p_in.opt()], outs=[PT.opt()])

            def conv_layer(TBL, out_dram, is_conv1):
                for g in meta["groups"]:
                    nch = g["nA"] + g["nB"]
                    msg = msgp.tile([P, NCHG_MAX, P], F16, tag="msg")
                    if g["nA"]:
                        nc.gpsimd.dma_gather(
                            out_ap=msg[:, 0:g["nA"], :],
                            in_ap=TBL[0:A_LIM, :],
                            idxs_ap=ei_s[:, g["colA"]:g["colA"] + g["nA"] * 8],
                            num_idxs=g["nA"] * P, num_idxs_reg=g["nA"] * P,
                            elem_size=P, single_packet=False)
                    if g["nB"]:
                        nc.gpsimd.dma_gather(
                            out_ap=msg[:, g["nA"]:nch, :],
                            in_ap=TBL[B_OFF:N_NODES, :],
                            idxs_ap=ei_s[:, g["colB"]:g["colB"] + g["nB"] * 8],
                            num_idxs=g["nB"] * P, num_idxs_reg=g["nB"] * P,
                            elem_size=P, single_packet=False)
                    for t in g["tiles"]:
                        m = min(P, NPC - t * P)
                        chunks = meta["tile_chunks"][t]
                        if is_conv1:
                            # psum [ch, d], seeded outer(b1, sqrtdeg)
                            ps = psA.tile([HID_CH, P], F32, tag="agg1",
                                          space="PSUM")
                            nc.tensor.matmul(
                                out=ps[:, 0:m], lhsT=b1_s[:],
                                rhs=sq_s[:, t * P:t * P + m],
                                start=True, stop=False)
                        else:
                            # psum [d, ch], seeded outer(sqrtdeg, b2)
                            ps = psA.tile([P, OUT_CH], F32, tag="agg2",
                                          space="PSUM")
                            nc.tensor.matmul(
                                out=ps[0:m, :],
                                lhsT=sq_s[:, t * P:t * P + m],
                                rhs=b2_s[:], start=True, stop=False)
                        for gc in chunks:
                            lc = gc - g["base"]
                            ind = indp.tile([P, P], F16, tag="ind")
                            nc.vector.tensor_scalar(
                                out=ind[:], in0=iota_s[:],
                                scalar1=el_s[:, gc:gc + 1],
                                scalar2=None,
                                op0=mybir.AluOpType.is_equal)
                            if is_conv1:
                                nc.tensor.matmul(
                                    out=ps[:, 0:m], lhsT=msg[:, lc, :],
                                    rhs=ind[:, 0:m],
                                    start=False, stop=False)
                            else:
                                nc.tensor.matmul(
                                    out=ps[0:m, :], lhsT=ind[:, 0:m],
                                    rhs=msg[:, lc, 0:OUT_CH],
                                    start=False, stop=False)
                        # self-loop: += p~[d] (resp. q~[d]) via identity
                        if is_conv1:
                            nc.tensor.matmul(
                                out=ps[:, 0:m], lhsT=p_keep[0:m, t, :],
                                rhs=ident_s[0:m, 0:m],
                                start=False, stop=True)
                            hT = evac.tile([HID_CH, P], F16, tag="hT")
                            nc.scalar.activation(
                                out=hT[:, 0:m], in_=ps[:, 0:m],
                                func=mybir.ActivationFunctionType.Relu)
                            psq = psB.tile([P, HID_CH], F32, tag="pp",
                                           space="PSUM")
                            nc.tensor.matmul(out=psq[0:m, 0:OUT_CH],
                                             lhsT=hT[:, 0:m], rhs=W2_s[:],
                                             start=True, stop=True)
                            qsb = evac.tile([P, P], F16, tag="qev")
                            nc.vector.memset(qsb[:, OUT_CH:P], 0)
                            nc.scalar.activation(
                                out=qsb[0:m, 0:OUT_CH], in_=psq[0:m, 0:OUT_CH],
                                func=mybir.ActivationFunctionType.Copy,
                                scale=dk2_s[0:m, t:t + 1])  # psq slice
                            nc.scalar.copy(out=q_keep[0:m, t, :],
                                           in_=qsb[0:m, 0:OUT_CH])
                            nc.sync.dma_start(
                                out=out_dram[t * P:t * P + m, :],
                                in_=qsb[0:m, :])
                        else:
                            nc.tensor.matmul(
                                out=ps[0:m, :], lhsT=ident_s[0:m, 0:m],
                                rhs=q_keep[0:m, t, :],
                                start=False, stop=True)
                            zsb = evac.tile([P, P], F16, tag="qev")
                            nc.vector.memset(zsb[:, OUT_CH:P], 0)
                            nc.scalar.activation(
                                out=zsb[0:m, 0:OUT_CH], in_=ps[0:m, :],
                                func=mybir.ActivationFunctionType.Copy,
                                scale=dk1_s[0:m, t:t + 1])
                            nc.scalar.copy(out=z_keep[0:m, t, :],
                                           in_=zsb[0:m, 0:OUT_CH])
                            nc.sync.dma_start(
                                out=out_dram[t * P:t * P + m, :],
                                in_=zsb[0:m, :])

            conv_layer(PT, q_in, True)
            nc.gpsimd.collective_compute(
                "AllGather", mybir.AluOpType.bypass, replica_groups=RG,
                ins=[q_in.opt()], outs=[QT.opt()])
            conv_layer(QT, z_in, False)
            nc.gpsimd.collective_compute(
                "AllGather", mybir.AluOpType.bypass, replica_groups=RG,
                ins=[z_in.opt()], outs=[ZT.opt()])

            # ---- decode: za via PE selection from z_keep, zb via
            # transposed dma_gather from ZT; dot = DVE mult + PE reduce
            for b in meta["lbatches"]:
                nch = b["nch"]
                zbT = decp.tile([P, LBATCH * P], F16, tag="zbT")
                av = (0, A_LIM) if b["view"] == 0 else (B_OFF, N_NODES)
                nc.gpsimd.dma_gather(
                    out_ap=zbT[:, 0:nch * P].rearrange(
                        "p (a b) -> p a b", a=1),
                    in_ap=ZT[av[0]:av[1], :],
                    idxs_ap=li_s[:, b["base"] * 8:(b["base"] + nch) * 8],
                    num_idxs=nch * P, num_idxs_reg=nch * P,
                    elem_size=P, single_packet=False, transpose=True)
                # ACT firewall: don't let DVE read dma_gather-written SBUF
                zb2 = decp.tile([P, LBATCH * P], F16, tag="zb2")
                nc.scalar.copy(out=zb2[0:OUT_CH, 0:nch * P],
                               in_=zbT[0:OUT_CH, 0:nch * P])
                la_t = decp.tile([P, LBATCH * P], F16, tag="la")
                nc.sync.dma_start(
                    out=la_t[:, 0:nch * P],
                    in_=laloc[:, b["base"] * P:(b["base"] + nch) * P])
                for ci in range(nch):
                    t = b["tiles"][ci]
                    m = min(P, NPC - t * P)
                    sel = indp.tile([P, P], F16, tag="sel")
                    nc.vector.tensor_tensor(
                        out=sel[:], in0=pmat_s[:],
                        in1=la_t[:, ci * P:ci * P + P],
                        op=mybir.AluOpType.is_equal)
                    psa = psA.tile([HID_CH, P], F32, tag="agg1",
                                   space="PSUM")
                    nc.tensor.matmul(out=psa[0:OUT_CH, :],
                                     lhsT=z_keep[0:m, t, :],
                                     rhs=sel[0:m, :], start=True, stop=True)
                    scr = indp.tile([OUT_CH, P], F16, tag="scr")
                    nc.vector.tensor_tensor(
                        out=scr[:], in0=psa[0:OUT_CH, :],
                        in1=zb2[0:OUT_CH, ci * P:ci * P + P],
                        op=mybir.AluOpType.mult)
                    psl = psB.tile([P, HID_CH], F32, tag="pp",
                                   space="PSUM")
                    nc.tensor.matmul(out=psl[:, 0:1], lhsT=scr[:],
                                     rhs=ones_s[0:OUT_CH, :],
                                     start=True, stop=True)
                    cc = b["base"] + ci
                    nc.scalar.copy(out=logit_sb[:, cc:cc + 1], in_=psl[:, 0:1])
            nc.sync.dma_start(out=logits[:], in_=logit_sb[:])

    nc.compile()
    return nc


_CACHE = {}
TRACE = False          # set True (e.g. from test.py) to capture an NTFF trace
LAST_RESULT = None     # BassKernelResults of the most recent run


def kernel(**inputs):
    meta, cores, perms = _prepare(**inputs)
    key = (meta["TOT_CH"], meta["LCH"], meta["WC"], meta["WL"])
    if key not in _CACHE:
        _CACHE[key] = _build(meta)
    nc = _CACHE[key]
    names = ("xT", "W1h", "W2h", "b1row", "b2row", "sqrow", "dk1", "dk2",
             "eidx", "edloc", "lidx", "laloc")
    in_maps = [{n: c[n] for n in names} for c in cores]
    res = run_bass_kernel_spmd(nc, in_maps, core_ids=list(range(N_CORES)),
                               trace=TRACE)
    global LAST_RESULT
    LAST_RESULT = res
    out = np.empty(N_LABEL, np.float32)
    for k in range(N_CORES):
        vals = res.results[k]["logits"].T.ravel()
        perm = perms[k]
        m = perm >= 0
        out[perm[m]] = vals[m]
    return out


# revision 16
# speedup vs baseline: 1.1996x; 1.0291x over previous
"""GCN link predictor on 8 TRN2 NeuronCores (Bass/Tile) — v2.

Design notes (driven by HW profiling of the v1 baseline, 2.34 ms):
the bottleneck is the GPSIMD (Pool) engine generating SWDGE descriptors for
dma_gather at ~8 ns/index, blocking, with no faster indexed primitive on the
machine (ap_gather ~27 ns/idx, scatter_add wedges the device).  So v2
minimizes gather indices and keeps everything else off the Pool engine:

- Normalization refactor: out[d] = b + dinv[d] * sum_e table[src_e] with
  table rows pre-scaled by dinv[src] (x rows host-scaled; q~ scaled dinv^2
  at evacuation).  Indicators become PURE one-hot -> single-op is_equal on
  DVE, and PSUM is seeded with outer(bias, sqrt(deg)) so the dst-side dinv
  folds into the existing evacuation scale.
- Self-loops never enter the gather path: one identity matmul per dst tile
  adds p~[d] (resp. q~[d]) from SBUF-resident local tiles.
- Decode: label pairs sharded by core(a); the a-side z rows come from PE
  one-hot selection out of SBUF-resident local z tiles (overlaps the last
  AllGather); only the b-side uses dma_gather (transpose=True -> zbT
  [ch, pair]); dot product = DVE multiply + PE ones-reduction.

dma_gather indices are int16, so 40000-row tables are addressed through two
overlapping views: A = rows [0, 32768), B = rows [7232, 40000).
"""

import numpy as np

import concourse.bass as bass
import concourse.bacc as bacc
import concourse.mybir as mybir
import concourse.tile as tile
from concourse.bass_utils import run_bass_kernel_spmd

P = 128
N_NODES = 40000
IN_CH = 128
HID_CH = 128
OUT_CH = 64
N_LABEL = 200000
N_CORES = 8
NPC = N_NODES // N_CORES          # 5000 nodes per core
NT = (NPC + P - 1) // P           # 40 dst tiles per core (last has 8 nodes)
A_LIM = 32768                     # view A = rows [0, 32768)
B_OFF = N_NODES - A_LIM           # 7232; view B = rows [7232, 40000)
GROUP = 4                         # dst tiles per gather group
LBATCH = 32                       # decode chunks per gather batch

F16 = mybir.dt.float16
F32 = mybir.dt.float32
I16 = mybir.dt.int16


def _wrap16(flat):
    """dma_gather / index SBUF image: position n -> [n % 16, n // 16],
    replicated across the 8 groups of 16 partitions. [128, len/16] int16."""
    n = len(flat)
    assert n % 16 == 0
    grid = np.asarray(flat, np.int16).reshape(n // 16, 16).T
    return np.tile(grid, (8, 1))


def _prepare(x, edge_index, edge_label_index, W1, b1, W2, b2):
    src = np.asarray(edge_index[0], np.int64)
    dst = np.asarray(edge_index[1], np.int64)
    deg = (np.bincount(dst, minlength=N_NODES) + 1).astype(np.float64)
    dinv = (1.0 / np.sqrt(deg)).astype(np.float32)
    sqrtdeg = np.sqrt(deg).astype(np.float32)

    # ---- bucket edges by (core, tile), sorted by src within each bucket
    core_of = dst // NPC
    tloc = (dst % NPC) // P
    order = np.lexsort((src, tloc, core_of))
    s_src = src[order]
    s_dst = dst[order]
    key = core_of[order] * NT + tloc[order]
    starts = np.searchsorted(key, np.arange(N_CORES * NT))
    ends = np.searchsorted(key, np.arange(N_CORES * NT) + 1)

    cnt = (ends - starts).reshape(N_CORES, NT)
    fA = np.empty((N_CORES, NT), np.int64)   # forced-A (< B_OFF)
    fB = np.empty((N_CORES, NT), np.int64)   # forced-B (>= A_LIM)
    for k in range(N_CORES):
        for t in range(NT):
            b = k * NT + t
            ss = s_src[starts[b]:ends[b]]
            fA[k, t] = np.searchsorted(ss, B_OFF)
            fB[k, t] = len(ss) - np.searchsorted(ss, A_LIM)
    NCA = np.maximum(1, (fA.max(axis=0) + P - 1) // P)          # [NT]
    nA = np.minimum(cnt - fB, P * NCA[None, :])                 # [cores, NT]
    nA = np.maximum(nA, 0)
    cntB = cnt - nA
    NCB = (cntB.max(axis=0) + P - 1) // P                       # [NT]

    groups = []
    tile_chunks = {}
    gbase = 0
    col = 0
    for g0 in range(0, NT, GROUP):
        ts = list(range(g0, min(g0 + GROUP, NT)))
        gnA = int(NCA[ts].sum())
        gnB = int(NCB[ts].sum())
        groups.append(dict(tiles=ts, base=gbase, nA=gnA, nB=gnB,
                           colA=col, colB=col + gnA * 8))
        ca = gbase
        cb = gbase + gnA
        for t in ts:
            tile_chunks[t] = (list(range(ca, ca + int(NCA[t])))
                              + list(range(cb, cb + int(NCB[t]))))
            ca += int(NCA[t])
            cb += int(NCB[t])
        gbase += gnA + gnB
        col += (gnA + gnB) * 8
    TOT_CH = gbase
    WC = col

    # ---- per-core conv arrays: gather indices + dst-local one-hot columns
    cores = []
    for k in range(N_CORES):
        eidx = np.zeros((P, WC), np.int16)
        edloc = np.full((P, TOT_CH), -1.0, np.float32)  # -1 -> no is_eq match
        for g in groups:
            flatA = []
            flatB = []
            for t in g["tiles"]:
                b = k * NT + t
                ss = s_src[starts[b]:ends[b]]
                dd = s_dst[starts[b]:ends[b]]
                na = int(nA[k, t])
                la = np.full(int(NCA[t]) * P, -1.0, np.float32)
                ia = np.zeros(int(NCA[t]) * P, np.int64)
                ia[:na] = ss[:na]
                la[:na] = (dd[:na] - k * NPC - t * P).astype(np.float32)
                lb_ = np.full(int(NCB[t]) * P, -1.0, np.float32)
                ib = np.zeros(int(NCB[t]) * P, np.int64)
                nb = int(cntB[k, t])
                ib[:nb] = ss[na:na + nb] - B_OFF
                lb_[:nb] = (dd[na:na + nb] - k * NPC - t * P).astype(
                    np.float32)
                flatA.append((ia, la))
                flatB.append((ib, lb_))
            ia = np.concatenate([f[0] for f in flatA])
            ib = np.concatenate([f[0] for f in flatB])
            locs = np.concatenate([f[1] for f in flatA]
                                  + [f[1] for f in flatB])
            if len(ia):
                eidx[:, g["colA"]:g["colA"] + len(ia) // 16] = _wrap16(ia)
            if len(ib):
                eidx[:, g["colB"]:g["colB"] + len(ib) // 16] = _wrap16(ib)
            nch = g["nA"] + g["nB"]
            edloc[:, g["base"]:g["base"] + nch] = locs.reshape(nch, P).T
        cores.append(dict(eidx=eidx, edloc=edloc))

    # ---- decode prep: pairs sharded by core(a), grouped by (a_tile, b_view)
    la_all = np.asarray(edge_label_index[0], np.int64)
    lb_all = np.asarray(edge_label_index[1], np.int64)
    owner = la_all // NPC
    # per (core, a_tile, view) counts to find shared padded chunk counts
    atile = (la_all % NPC) // P
    bview = (lb_all >= A_LIM).astype(np.int64)  # 0 -> A view, 1 -> B view
    cntd = np.zeros((N_CORES, NT, 2), np.int64)
    for k in range(N_CORES):
        m = owner == k
        np.add.at(cntd[k], (atile[m], bview[m]), 1)
    NCD = (cntd.max(axis=0) + P - 1) // P                       # [NT, 2]
    # chunk layout: all view-A chunks (tile-major), then all view-B chunks
    chunksA = []
    chunksB = []
    for t in range(NT):
        for c in range(int(NCD[t, 0])):
            chunksA.append(t)
        for c in range(int(NCD[t, 1])):
            chunksB.append(t)
    LCH = len(chunksA) + len(chunksB)
    lbatches = []
    for v, chs, base in ((0, chunksA, 0), (1, chunksB, len(chunksA))):
        for c0 in range(0, len(chs), LBATCH):
            nch = min(LBATCH, len(chs) - c0)
            lbatches.append(dict(view=v, base=base + c0, nch=nch,
                                 tiles=chs[c0:c0 + nch]))
    WL = LCH * 8

    perms = []
    for k in range(N_CORES):
        m = owner == k
        ga, gb, gidx = la_all[m], lb_all[m], np.nonzero(m)[0]
        at, bv = atile[m], bview[m]
        o = np.lexsort((gb, bv, at))
        ga, gb, gidx, at, bv = ga[o], gb[o], gidx[o], at[o], bv[o]
        lidx = np.zeros((P, WL), np.int16)
        laloc = np.full(LCH * P, -1.0, np.float32)
        perm = np.full(LCH * P, -1, np.int64)
        cbase = {0: 0, 1: len(chunksA)}
        coff = {0: 0, 1: 0}
        for t in range(NT):
            for v in (0, 1):
                mm = (at == t) & (bv == v)
                pa, pb, pi = ga[mm], gb[mm], gidx[mm]
                ncap = int(NCD[t, v]) * P
                assert len(pa) <= ncap
                ids = np.zeros(ncap, np.int64)
                ids[:len(pb)] = pb - (0 if v == 0 else B_OFF)
                start = cbase[v] + coff[v]
                lidx[:, start * 8:(start + int(NCD[t, v])) * 8] = \
                    _wrap16(ids)
                sl = slice(start * P, start * P + len(pa))
                laloc[start * P:(start + int(NCD[t, v])) * P][:len(pa)] = \
                    (pa - k * NPC - t * P).astype(np.float32)
                perm[sl] = pi
                coff[v] += int(NCD[t, v])
        # laloc broadcast image: [128 partitions, LCH*128] fp16, value =
        # a_loc of the pair in that column (same in every partition)
        lab = np.broadcast_to(laloc[None, :], (P, LCH * P)).astype(np.float16)
        cores[k]["lidx"] = lidx
        cores[k]["laloc"] = np.ascontiguousarray(lab)
        perms.append(perm)

    # ---- dense inputs per core
    x = np.asarray(x, np.float32)
    for k in range(N_CORES):
        xk = x[k * NPC:(k + 1) * NPC] * dinv[k * NPC:(k + 1) * NPC, None]
        cores[k]["xT"] = np.ascontiguousarray(xk.T).astype(np.float16)
        cores[k]["W1h"] = np.asarray(W1, np.float32).astype(np.float16)
        cores[k]["W2h"] = np.asarray(W2, np.float32).astype(np.float16)
        cores[k]["b1row"] = np.asarray(b1, np.float32).astype(
            np.float16).reshape(1, HID_CH)
        cores[k]["b2row"] = np.asarray(b2, np.float32).astype(
            np.float16).reshape(1, OUT_CH)
        sq = np.zeros((1, NT * P), np.float16)
        sq[0, :NPC] = sqrtdeg[k * NPC:(k + 1) * NPC]
        cores[k]["sqrow"] = sq
        dk1 = np.ones((P, NT), np.float32)
        dk2 = np.ones((P, NT), np.float32)
        dv = dinv[k * NPC:(k + 1) * NPC]
        for t in range(NT):
            m = min(P, NPC - t * P)
            dk1[:m, t] = dv[t * P:t * P + m]
            dk2[:m, t] = dv[t * P:t * P + m] ** 2
        cores[k]["dk1"] = dk1
        cores[k]["dk2"] = dk2

    meta = dict(groups=groups, tile_chunks=tile_chunks, TOT_CH=TOT_CH,
                WC=WC, lbatches=lbatches, LCH=LCH, WL=WL,
                NCD=[[int(v) for v in row] for row in NCD])
    return meta, cores, perms


def _build(meta):
    TOT_CH, WC, LCH, WL = (meta["TOT_CH"], meta["WC"],
                           meta["LCH"], meta["WL"])
    NCHG_MAX = max(g["nA"] + g["nB"] for g in meta["groups"])

    nc = bacc.Bacc("TRN2", target_bir_lowering=False, debug=False,
                   num_devices=N_CORES)
    xT = nc.dram_tensor("xT", [P, NPC], F16, kind="ExternalInput")
    W1h = nc.dram_tensor("W1h", [P, HID_CH], F16, kind="ExternalInput")
    W2h = nc.dram_tensor("W2h", [P, OUT_CH], F16, kind="ExternalInput")
    b1row = nc.dram_tensor("b1row", [1, HID_CH], F16, kind="ExternalInput")
    b2row = nc.dram_tensor("b2row", [1, OUT_CH], F16, kind="ExternalInput")
    sqrow = nc.dram_tensor("sqrow", [1, NT * P], F16, kind="ExternalInput")
    dk1 = nc.dram_tensor("dk1", [P, NT], F32, kind="ExternalInput")
    dk2 = nc.dram_tensor("dk2", [P, NT], F32, kind="ExternalInput")
    eidx = nc.dram_tensor("eidx", [P, WC], I16, kind="ExternalInput")
    edloc = nc.dram_tensor("edloc", [P, TOT_CH], F32, kind="ExternalInput")
    lidx = nc.dram_tensor("lidx", [P, WL], I16, kind="ExternalInput")
    laloc = nc.dram_tensor("laloc", [P, LCH * P], F16, kind="ExternalInput")
    logits = nc.dram_tensor("logits", [P, LCH], F32, kind="ExternalOutput")

    RG = [list(range(N_CORES))]

    with tile.TileContext(nc) as tc:
        with tc.tile_pool(name="const", bufs=1) as cpool, \
             tc.tile_pool(name="msgp", bufs=4) as msgp, \
             tc.tile_pool(name="indp", bufs=4) as indp, \
             tc.tile_pool(name="evac", bufs=3) as evac, \
             tc.tile_pool(name="decp", bufs=3) as decp, \
             tc.tile_pool(name="psA", bufs=2, space="PSUM") as psA, \
             tc.tile_pool(name="psB", bufs=2, space="PSUM") as psB, \
             tc.tile_pool(name="dram", bufs=1, space="DRAM") as dram:

            # constants into SBUF
            xT_s = cpool.tile([P, NPC], F16)
            W1_s = cpool.tile([P, HID_CH], F16)
            W2_s = cpool.tile([P, OUT_CH], F16)
            b1_s = cpool.tile([1, HID_CH], F16)
            b2_s = cpool.tile([1, OUT_CH], F16)
            sq_s = cpool.tile([1, NT * P], F16)
            dk1_s = cpool.tile([P, NT], F32)
            dk2_s = cpool.tile([P, NT], F32)
            ei_s = cpool.tile([P, WC], I16)
            el_s = cpool.tile([P, TOT_CH], F32)
            li_s = cpool.tile([P, WL], I16)
            iota_s = cpool.tile([P, P], F16)
            pcol_s = cpool.tile([P, 1], F32)
            ident_s = cpool.tile([P, P], F16)
            ones_s = cpool.tile([P, 1], F16)
            p_keep = cpool.tile([P, NT, HID_CH], F16)
            q_keep = cpool.tile([P, NT, OUT_CH], F16)
            z_keep = cpool.tile([P, NT, OUT_CH], F16)
            logit_sb = cpool.tile([P, LCH], F32)
            nc.sync.dma_start(out=xT_s[:], in_=xT[:])
            nc.sync.dma_start(out=W1_s[:], in_=W1h[:])
            nc.sync.dma_start(out=W2_s[:], in_=W2h[:])
            nc.sync.dma_start(out=b1_s[:], in_=b1row[:])
            nc.sync.dma_start(out=b2_s[:], in_=b2row[:])
            nc.sync.dma_start(out=sq_s[:], in_=sqrow[:])
            nc.sync.dma_start(out=dk1_s[:], in_=dk1[:])
            nc.sync.dma_start(out=dk2_s[:], in_=dk2[:])
            nc.sync.dma_start(out=ei_s[:], in_=eidx[:])
            nc.sync.dma_start(out=el_s[:], in_=edloc[:])
            nc.sync.dma_start(out=li_s[:], in_=lidx[:])
            nc.vector.memset(ones_s[:], 1.0)
            nc.gpsimd.iota(iota_s[:], pattern=[[1, P]], base=0,
                           channel_multiplier=0,
                           allow_small_or_imprecise_dtypes=True)
            nc.gpsimd.iota(pcol_s[:], pattern=[[0, 1]], base=0,
                           channel_multiplier=1,
                           allow_small_or_imprecise_dtypes=True)
            pmat_s = cpool.tile([P, P], F16)
            nc.gpsimd.iota(pmat_s[:], pattern=[[0, P]], base=0,
                           channel_multiplier=1,
                           allow_small_or_imprecise_dtypes=True)
            nc.vector.tensor_scalar(
                out=ident_s[:], in0=iota_s[:], scalar1=pcol_s[:],
                scalar2=None, op0=mybir.AluOpType.is_equal)

            p_in = dram.tile([NPC, HID_CH], F16)
            PT = dram.tile([N_NODES, HID_CH], F16, addr_space="Shared")
            q_in = dram.tile([NPC, P], F16)
            QT = dram.tile([N_NODES, P], F16, addr_space="Shared")
            z_in = dram.tile([NPC, P], F16)
            ZT = dram.tile([N_NODES, P], F16, addr_space="Shared")

            # ---- stage 1: p~ = (x*dinv) @ W1, per tile; keep + publish
            for t in range(NT):
                m = min(P, NPC - t * P)
                psum_p = psB.tile([P, HID_CH], F32, tag="pp", space="PSUM")
                nc.tensor.matmul(out=psum_p[0:m, :],
                                 lhsT=xT_s[:, t * P:t * P + m],
                                 rhs=W1_s[:], start=True, stop=True)
                nc.scalar.copy(out=p_keep[0:m, t, :], in_=psum_p[0:m, :])
            nc.sync.dma_start(
                out=p_in[0:(NT - 1) * P, :].rearrange(
                    "(t p) c -> p t c", p=P),
                in_=p_keep[:, 0:NT - 1, :])
            nc.sync.dma_start(out=p_in[(NT - 1) * P:NPC, :],
                              in_=p_keep[0:NPC - (NT - 1) * P, NT - 1, :])

            nc.gpsimd.collective_compute(
                "AllGather", mybir.AluOpType.bypass, replica_groups=RG,
                ins=[p_in.opt()], outs=[PT.opt()])

            def conv_layer(TBL, out_dram, is_conv1):
                keep = q_keep if is_conv1 else z_keep
                for g in meta["groups"]:
                    nch = g["nA"] + g["nB"]
                    msg = msgp.tile([P, NCHG_MAX, P], F16, tag="msg")
                    if g["nA"]:
                        nc.gpsimd.dma_gather(
                            out_ap=msg[:, 0:g["nA"], :],
                            in_ap=TBL[0:A_LIM, :],
                            idxs_ap=ei_s[:, g["colA"]:g["colA"] + g["nA"] * 8],
                            num_idxs=g["nA"] * P, num_idxs_reg=g["nA"] * P,
                            elem_size=P, single_packet=False)
                    if g["nB"]:
                        nc.gpsimd.dma_gather(
                            out_ap=msg[:, g["nA"]:nch, :],
                            in_ap=TBL[B_OFF:N_NODES, :],
                            idxs_ap=ei_s[:, g["colB"]:g["colB"] + g["nB"] * 8],
                            num_idxs=g["nB"] * P, num_idxs_reg=g["nB"] * P,
                            elem_size=P, single_packet=False)
                    for t in g["tiles"]:
                        m = min(P, NPC - t * P)
                        chunks = meta["tile_chunks"][t]
                        if is_conv1:
                            # psum [ch, d], seeded outer(b1, sqrtdeg)
                            ps = psA.tile([HID_CH, P], F32, tag="agg1",
                                          space="PSUM")
                            nc.tensor.matmul(
                                out=ps[:, 0:m], lhsT=b1_s[:],
                                rhs=sq_s[:, t * P:t * P + m],
                                start=True, stop=False)
                        else:
                            # psum [d, ch], seeded outer(sqrtdeg, b2)
                            ps = psA.tile([P, OUT_CH], F32, tag="agg2",
                                          space="PSUM")
                            nc.tensor.matmul(
                                out=ps[0:m, :],
                                lhsT=sq_s[:, t * P:t * P + m],
                                rhs=b2_s[:], start=True, stop=False)
                        for gc in chunks:
                            lc = gc - g["base"]
                            ind = indp.tile([P, P], F16, tag="ind")
                            nc.vector.tensor_scalar(
                                out=ind[:], in0=iota_s[:],
                                scalar1=el_s[:, gc:gc + 1],
                                scalar2=None,
                                op0=mybir.AluOpType.is_equal)
                            if is_conv1:
                                nc.tensor.matmul(
                                    out=ps[:, 0:m], lhsT=msg[:, lc, :],
                                    rhs=ind[:, 0:m],
                                    start=False, stop=False)
                            else:
                                nc.tensor.matmul(
                                    out=ps[0:m, :], lhsT=ind[:, 0:m],
                                    rhs=msg[:, lc, 0:OUT_CH],
                                    start=False, stop=False)
                        # self-loop: += p~[d] (resp. q~[d]) via identity
                        if is_conv1:
                            nc.tensor.matmul(
                                out=ps[:, 0:m], lhsT=p_keep[0:m, t, :],
                                rhs=ident_s[0:m, 0:m],
                                start=False, stop=True)
                            hT = evac.tile([HID_CH, P], F16, tag="hT")
                            nc.scalar.activation(
                                out=hT[:, 0:m], in_=ps[:, 0:m],
                                func=mybir.ActivationFunctionType.Relu)
                            psq = psB.tile([P, HID_CH], F32, tag="pp",
                                           space="PSUM")
                            nc.tensor.matmul(out=psq[0:m, 0:OUT_CH],
                                             lhsT=hT[:, 0:m], rhs=W2_s[:],
                                             start=True, stop=True)
                            nc.scalar.activation(
                                out=q_keep[0:m, t, :],
                                in_=psq[0:m, 0:OUT_CH],
                                func=mybir.ActivationFunctionType.Copy,
                                scale=dk2_s[0:m, t:t + 1])
                        else:
                            nc.tensor.matmul(
                                out=ps[0:m, :], lhsT=ident_s[0:m, 0:m],
                                rhs=q_keep[0:m, t, :],
                                start=False, stop=True)
                            nc.scalar.activation(
                                out=z_keep[0:m, t, :], in_=ps[0:m, :],
                                func=mybir.ActivationFunctionType.Copy,
                                scale=dk1_s[0:m, t:t + 1])

            conv_layer(PT, q_in, True)
            nc.sync.dma_start(
                out=q_in[0:(NT - 1) * P, 0:OUT_CH].rearrange(
                    "(t p) c -> p t c", p=P),
                in_=q_keep[:, 0:NT - 1, :])
            nc.sync.dma_start(out=q_in[(NT - 1) * P:NPC, 0:OUT_CH],
                              in_=q_keep[0:NPC - (NT - 1) * P, NT - 1, :])
            nc.gpsimd.collective_compute(
                "AllGather", mybir.AluOpType.bypass, replica_groups=RG,
                ins=[q_in.opt()], outs=[QT.opt()])
            conv_layer(QT, z_in, False)
            nc.sync.dma_start(
                out=z_in[0:(NT - 1) * P, 0:OUT_CH].rearrange(
                    "(t p) c -> p t c", p=P),
                in_=z_keep[:, 0:NT - 1, :])
            nc.sync.dma_start(out=z_in[(NT - 1) * P:NPC, 0:OUT_CH],
                              in_=z_keep[0:NPC - (NT - 1) * P, NT - 1, :])
            nc.gpsimd.collective_compute(
                "AllGather", mybir.AluOpType.bypass, replica_groups=RG,
                ins=[z_in.opt()], outs=[ZT.opt()])

            # ---- decode: za via PE selection from z_keep, zb via
            # transposed dma_gather from ZT; dot = DVE mult + PE reduce
            for b in meta["lbatches"]:
                nch = b["nch"]
                zbT = decp.tile([P, LBATCH * P], F16, tag="zbT")
                av = (0, A_LIM) if b["view"] == 0 else (B_OFF, N_NODES)
                nc.gpsimd.dma_gather(
                    out_ap=zbT[:, 0:nch * P].rearrange(
                        "p (a b) -> p a b", a=1),
                    in_ap=ZT[av[0]:av[1], :],
                    idxs_ap=li_s[:, b["base"] * 8:(b["base"] + nch) * 8],
                    num_idxs=nch * P, num_idxs_reg=nch * P,
                    elem_size=P, single_packet=False, transpose=True)
                # ACT firewall: don't let DVE read dma_gather-written SBUF
                zb2 = decp.tile([P, LBATCH * P], F16, tag="zb2")
                nc.scalar.copy(out=zb2[0:OUT_CH, 0:nch * P],
                               in_=zbT[0:OUT_CH, 0:nch * P])
                la_t = decp.tile([P, LBATCH * P], F16, tag="la")
                nc.sync.dma_start(
                    out=la_t[:, 0:nch * P],
                    in_=laloc[:, b["base"] * P:(b["base"] + nch) * P])
                for ci in range(nch):
                    t = b["tiles"][ci]
                    m = min(P, NPC - t * P)
                    sel = indp.tile([P, P], F16, tag="sel")
                    nc.vector.tensor_tensor(
                        out=sel[:], in0=pmat_s[:],
                        in1=la_t[:, ci * P:ci * P + P],
                        op=mybir.AluOpType.is_equal)
                    psa = psA.tile([HID_CH, P], F32, tag="agg1",
                                   space="PSUM")
                    nc.tensor.matmul(out=psa[0:OUT_CH, :],
                                     lhsT=z_keep[0:m, t, :],
                                     rhs=sel[0:m, :], start=True, stop=True)
                    scr = indp.tile([OUT_CH, P], F16, tag="scr")
                    nc.vector.tensor_tensor(
                        out=scr[:], in0=psa[0:OUT_CH, :],
                        in1=zb2[0:OUT_CH, ci * P:ci * P + P],
                        op=mybir.AluOpType.mult)
                    psl = psB.tile([P, HID_CH], F32, tag="pp",
                                   space="PSUM")
                    nc.tensor.matmul(out=psl[:, 0:1], lhsT=scr[:],
                                     rhs=ones_s[0:OUT_CH, :],
                                     start=True, stop=True)
                    cc = b["base"] + ci
                    nc.scalar.copy(out=logit_sb[:, cc:cc + 1], in_=psl[:, 0:1])
            nc.sync.dma_start(out=logits[:], in_=logit_sb[:])

    nc.compile()
    return nc


_CACHE = {}
TRACE = False          # set True (e.g. from test.py) to capture an NTFF trace
LAST_RESULT = None     # BassKernelResults of the most recent run


def kernel(**inputs):
    meta, cores, perms = _prepare(**inputs)
    key = (meta["TOT_CH"], meta["LCH"], meta["WC"], meta["WL"])
    if key not in _CACHE:
        _CACHE[key] = _build(meta)
    nc = _CACHE[key]
    names = ("xT", "W1h", "W2h", "b1row", "b2row", "sqrow", "dk1", "dk2",
             "eidx", "edloc", "lidx", "laloc")
    in_maps = [{n: c[n] for n in names} for c in cores]
    res = run_bass_kernel_spmd(nc, in_maps, core_ids=list(range(N_CORES)),
                               trace=TRACE)
    global LAST_RESULT
    LAST_RESULT = res
    out = np.empty(N_LABEL, np.float32)
    for k in range(N_CORES):
        vals = res.results[k]["logits"].T.ravel()
        perm = perms[k]
        m = perm >= 0
        out[perm[m]] = vals[m]
    return out


# revision 17
# speedup vs baseline: 1.2150x; 1.0128x over previous
"""GCN link predictor on 8 TRN2 NeuronCores (Bass/Tile) — v2.

Design notes (driven by HW profiling of the v1 baseline, 2.34 ms):
the bottleneck is the GPSIMD (Pool) engine generating SWDGE descriptors for
dma_gather at ~8 ns/index, blocking, with no faster indexed primitive on the
machine (ap_gather ~27 ns/idx, scatter_add wedges the device).  So v2
minimizes gather indices and keeps everything else off the Pool engine:

- Normalization refactor: out[d] = b + dinv[d] * sum_e table[src_e] with
  table rows pre-scaled by dinv[src] (x rows host-scaled; q~ scaled dinv^2
  at evacuation).  Indicators become PURE one-hot -> single-op is_equal on
  DVE, and PSUM is seeded with outer(bias, sqrt(deg)) so the dst-side dinv
  folds into the existing evacuation scale.
- Self-loops never enter the gather path: one identity matmul per dst tile
  adds p~[d] (resp. q~[d]) from SBUF-resident local tiles.
- Decode: label pairs sharded by core(a); the a-side z rows come from PE
  one-hot selection out of SBUF-resident local z tiles (overlaps the last
  AllGather); only the b-side uses dma_gather (transpose=True -> zbT
  [ch, pair]); dot product = DVE multiply + PE ones-reduction.

dma_gather indices are int16, so 40000-row tables are addressed through two
overlapping views: A = rows [0, 32768), B = rows [7232, 40000).
"""

import numpy as np

import concourse.bass as bass
import concourse.bacc as bacc
import concourse.mybir as mybir
import concourse.tile as tile
from concourse.bass_utils import run_bass_kernel_spmd

P = 128
N_NODES = 40000
IN_CH = 128
HID_CH = 128
OUT_CH = 64
N_LABEL = 200000
N_CORES = 8
NPC = N_NODES // N_CORES          # 5000 nodes per core
NT = (NPC + P - 1) // P           # 40 dst tiles per core (last has 8 nodes)
A_LIM = 32768                     # view A = rows [0, 32768)
B_OFF = N_NODES - A_LIM           # 7232; view B = rows [7232, 40000)
GROUP = 4                         # dst tiles per gather group
LBATCH = 32                       # decode chunks per gather batch

F16 = mybir.dt.float16
F32 = mybir.dt.float32
I16 = mybir.dt.int16


def _wrap16(flat):
    """dma_gather / index SBUF image: position n -> [n % 16, n // 16],
    replicated across the 8 groups of 16 partitions. [128, len/16] int16."""
    n = len(flat)
    assert n % 16 == 0
    grid = np.asarray(flat, np.int16).reshape(n // 16, 16).T
    return np.tile(grid, (8, 1))


def _prepare(x, edge_index, edge_label_index, W1, b1, W2, b2):
    src = np.asarray(edge_index[0], np.int64)
    dst = np.asarray(edge_index[1], np.int64)
    deg = (np.bincount(dst, minlength=N_NODES) + 1).astype(np.float64)
    dinv = (1.0 / np.sqrt(deg)).astype(np.float32)
    sqrtdeg = np.sqrt(deg).astype(np.float32)

    # ---- bucket edges by (core, tile), sorted by src within each bucket
    core_of = dst // NPC
    tloc = (dst % NPC) // P
    order = np.lexsort((src, tloc, core_of))
    s_src = src[order]
    s_dst = dst[order]
    key = core_of[order] * NT + tloc[order]
    starts = np.searchsorted(key, np.arange(N_CORES * NT))
    ends = np.searchsorted(key, np.arange(N_CORES * NT) + 1)

    cnt = (ends - starts).reshape(N_CORES, NT)
    fA = np.empty((N_CORES, NT), np.int64)   # forced-A (< B_OFF)
    fB = np.empty((N_CORES, NT), np.int64)   # forced-B (>= A_LIM)
    for k in range(N_CORES):
        for t in range(NT):
            b = k * NT + t
            ss = s_src[starts[b]:ends[b]]
            fA[k, t] = np.searchsorted(ss, B_OFF)
            fB[k, t] = len(ss) - np.searchsorted(ss, A_LIM)
    NCA = np.maximum(1, (fA.max(axis=0) + P - 1) // P)          # [NT]
    nA = np.minimum(cnt - fB, P * NCA[None, :])                 # [cores, NT]
    nA = np.maximum(nA, 0)
    cntB = cnt - nA
    NCB = (cntB.max(axis=0) + P - 1) // P                       # [NT]

    groups = []
    tile_chunks = {}
    gbase = 0
    col = 0
    for g0 in range(0, NT, GROUP):
        ts = list(range(g0, min(g0 + GROUP, NT)))
        gnA = int(NCA[ts].sum())
        gnB = int(NCB[ts].sum())
        groups.append(dict(tiles=ts, base=gbase, nA=gnA, nB=gnB,
                           colA=col, colB=col + gnA * 8))
        ca = gbase
        cb = gbase + gnA
        for t in ts:
            tile_chunks[t] = (list(range(ca, ca + int(NCA[t])))
                              + list(range(cb, cb + int(NCB[t]))))
            ca += int(NCA[t])
            cb += int(NCB[t])
        gbase += gnA + gnB
        col += (gnA + gnB) * 8
    TOT_CH = gbase
    WC = col

    # ---- per-core conv arrays: gather indices + dst-local one-hot columns
    cores = []
    for k in range(N_CORES):
        eidx = np.zeros((P, WC), np.int16)
        edloc = np.full((P, TOT_CH), -1.0, np.float32)  # -1 -> no is_eq match
        for g in groups:
            flatA = []
            flatB = []
            for t in g["tiles"]:
                b = k * NT + t
                ss = s_src[starts[b]:ends[b]]
                dd = s_dst[starts[b]:ends[b]]
                na = int(nA[k, t])
                la = np.full(int(NCA[t]) * P, -1.0, np.float32)
                ia = np.zeros(int(NCA[t]) * P, np.int64)
                ia[:na] = ss[:na]
                la[:na] = (dd[:na] - k * NPC - t * P).astype(np.float32)
                lb_ = np.full(int(NCB[t]) * P, -1.0, np.float32)
                ib = np.zeros(int(NCB[t]) * P, np.int64)
                nb = int(cntB[k, t])
                ib[:nb] = ss[na:na + nb] - B_OFF
                lb_[:nb] = (dd[na:na + nb] - k * NPC - t * P).astype(
                    np.float32)
                flatA.append((ia, la))
                flatB.append((ib, lb_))
            ia = np.concatenate([f[0] for f in flatA])
            ib = np.concatenate([f[0] for f in flatB])
            locs = np.concatenate([f[1] for f in flatA]
                                  + [f[1] for f in flatB])
            if len(ia):
                eidx[:, g["colA"]:g["colA"] + len(ia) // 16] = _wrap16(ia)
            if len(ib):
                eidx[:, g["colB"]:g["colB"] + len(ib) // 16] = _wrap16(ib)
            nch = g["nA"] + g["nB"]
            edloc[:, g["base"]:g["base"] + nch] = locs.reshape(nch, P).T
        cores.append(dict(eidx=eidx, edloc=edloc))

    # ---- decode prep: pairs sharded by core(a), grouped by (a_tile, b_view)
    la_all = np.asarray(edge_label_index[0], np.int64)
    lb_all = np.asarray(edge_label_index[1], np.int64)
    owner = la_all // NPC
    # per (core, a_tile, view) counts to find shared padded chunk counts
    atile = (la_all % NPC) // P
    bview = (lb_all >= A_LIM).astype(np.int64)  # 0 -> A view, 1 -> B view
    cntd = np.zeros((N_CORES, NT, 2), np.int64)
    for k in range(N_CORES):
        m = owner == k
        np.add.at(cntd[k], (atile[m], bview[m]), 1)
    NCD = (cntd.max(axis=0) + P - 1) // P                       # [NT, 2]
    # chunk layout: all view-A chunks (tile-major), then all view-B chunks
    chunksA = []
    chunksB = []
    for t in range(NT):
        for c in range(int(NCD[t, 0])):
            chunksA.append(t)
        for c in range(int(NCD[t, 1])):
            chunksB.append(t)
    LCH = len(chunksA) + len(chunksB)
    lbatches = []
    for v, chs, base in ((0, chunksA, 0), (1, chunksB, len(chunksA))):
        c0 = 0
        while c0 < len(chs):
            rem = len(chs) - c0
            # keep the final batches small so the post-AG3 tail is short
            nch = min(LBATCH if rem > LBATCH else max(8, rem // 2), rem)
            lbatches.append(dict(view=v, base=base + c0, nch=nch,
                                 tiles=chs[c0:c0 + nch]))
            c0 += nch
    WL = LCH * 8

    perms = []
    for k in range(N_CORES):
        m = owner == k
        ga, gb, gidx = la_all[m], lb_all[m], np.nonzero(m)[0]
        at, bv = atile[m], bview[m]
        o = np.lexsort((gb, bv, at))
        ga, gb, gidx, at, bv = ga[o], gb[o], gidx[o], at[o], bv[o]
        lidx = np.zeros((P, WL), np.int16)
        laloc = np.full(LCH * P, -1.0, np.float32)
        perm = np.full(LCH * P, -1, np.int64)
        cbase = {0: 0, 1: len(chunksA)}
        coff = {0: 0, 1: 0}
        for t in range(NT):
            for v in (0, 1):
                mm = (at == t) & (bv == v)
                pa, pb, pi = ga[mm], gb[mm], gidx[mm]
                ncap = int(NCD[t, v]) * P
                assert len(pa) <= ncap
                ids = np.zeros(ncap, np.int64)
                ids[:len(pb)] = pb - (0 if v == 0 else B_OFF)
                start = cbase[v] + coff[v]
                lidx[:, start * 8:(start + int(NCD[t, v])) * 8] = \
                    _wrap16(ids)
                sl = slice(start * P, start * P + len(pa))
                laloc[start * P:(start + int(NCD[t, v])) * P][:len(pa)] = \
                    (pa - k * NPC - t * P).astype(np.float32)
                perm[sl] = pi
                coff[v] += int(NCD[t, v])
        # laloc broadcast image: [128 partitions, LCH*128] fp16, value =
        # a_loc of the pair in that column (same in every partition)
        lab = np.broadcast_to(laloc[None, :], (P, LCH * P)).astype(np.float16)
        cores[k]["lidx"] = lidx
        cores[k]["laloc"] = np.ascontiguousarray(lab)
        perms.append(perm)

    # ---- dense inputs per core
    x = np.asarray(x, np.float32)
    for k in range(N_CORES):
        xk = x[k * NPC:(k + 1) * NPC] * dinv[k * NPC:(k + 1) * NPC, None]
        cores[k]["xT"] = np.ascontiguousarray(xk.T).astype(np.float16)
        cores[k]["W1h"] = np.asarray(W1, np.float32).astype(np.float16)
        cores[k]["W2h"] = np.asarray(W2, np.float32).astype(np.float16)
        cores[k]["b1row"] = np.asarray(b1, np.float32).astype(
            np.float16).reshape(1, HID_CH)
        cores[k]["b2row"] = np.asarray(b2, np.float32).astype(
            np.float16).reshape(1, OUT_CH)
        sq = np.zeros((1, NT * P), np.float16)
        sq[0, :NPC] = sqrtdeg[k * NPC:(k + 1) * NPC]
        cores[k]["sqrow"] = sq
        dk1 = np.ones((P, NT), np.float32)
        dk2 = np.ones((P, NT), np.float32)
        dv = dinv[k * NPC:(k + 1) * NPC]
        for t in range(NT):
            m = min(P, NPC - t * P)
            dk1[:m, t] = dv[t * P:t * P + m]
            dk2[:m, t] = dv[t * P:t * P + m] ** 2
        cores[k]["dk1"] = dk1
        cores[k]["dk2"] = dk2

    meta = dict(groups=groups, tile_chunks=tile_chunks, TOT_CH=TOT_CH,
                WC=WC, lbatches=lbatches, LCH=LCH, WL=WL,
                NCD=[[int(v) for v in row] for row in NCD])
    return meta, cores, perms


def _build(meta):
    TOT_CH, WC, LCH, WL = (meta["TOT_CH"], meta["WC"],
                           meta["LCH"], meta["WL"])
    NCHG_MAX = max(g["nA"] + g["nB"] for g in meta["groups"])

    nc = bacc.Bacc("TRN2", target_bir_lowering=False, debug=False,
                   num_devices=N_CORES)
    xT = nc.dram_tensor("xT", [P, NPC], F16, kind="ExternalInput")
    W1h = nc.dram_tensor("W1h", [P, HID_CH], F16, kind="ExternalInput")
    W2h = nc.dram_tensor("W2h", [P, OUT_CH], F16, kind="ExternalInput")
    b1row = nc.dram_tensor("b1row", [1, HID_CH], F16, kind="ExternalInput")
    b2row = nc.dram_tensor("b2row", [1, OUT_CH], F16, kind="ExternalInput")
    sqrow = nc.dram_tensor("sqrow", [1, NT * P], F16, kind="ExternalInput")
    dk1 = nc.dram_tensor("dk1", [P, NT], F32, kind="ExternalInput")
    dk2 = nc.dram_tensor("dk2", [P, NT], F32, kind="ExternalInput")
    eidx = nc.dram_tensor("eidx", [P, WC], I16, kind="ExternalInput")
    edloc = nc.dram_tensor("edloc", [P, TOT_CH], F32, kind="ExternalInput")
    lidx = nc.dram_tensor("lidx", [P, WL], I16, kind="ExternalInput")
    laloc = nc.dram_tensor("laloc", [P, LCH * P], F16, kind="ExternalInput")
    logits = nc.dram_tensor("logits", [P, LCH], F32, kind="ExternalOutput")

    RG = [list(range(N_CORES))]

    with tile.TileContext(nc) as tc:
        with tc.tile_pool(name="const", bufs=1) as cpool, \
             tc.tile_pool(name="msgp", bufs=4) as msgp, \
             tc.tile_pool(name="indp", bufs=6) as indp, \
             tc.tile_pool(name="evac", bufs=3) as evac, \
             tc.tile_pool(name="decp", bufs=3) as decp, \
             tc.tile_pool(name="psA", bufs=3, space="PSUM") as psA, \
             tc.tile_pool(name="psB", bufs=2, space="PSUM") as psB, \
             tc.tile_pool(name="dram", bufs=1, space="DRAM") as dram:

            # constants into SBUF
            xT_s = cpool.tile([P, NPC], F16)
            W1_s = cpool.tile([P, HID_CH], F16)
            W2_s = cpool.tile([P, OUT_CH], F16)
            b1_s = cpool.tile([1, HID_CH], F16)
            b2_s = cpool.tile([1, OUT_CH], F16)
            sq_s = cpool.tile([1, NT * P], F16)
            dk1_s = cpool.tile([P, NT], F32)
            dk2_s = cpool.tile([P, NT], F32)
            ei_s = cpool.tile([P, WC], I16)
            el_s = cpool.tile([P, TOT_CH], F32)
            li_s = cpool.tile([P, WL], I16)
            iota_s = cpool.tile([P, P], F16)
            pcol_s = cpool.tile([P, 1], F32)
            ident_s = cpool.tile([P, P], F16)
            ones_s = cpool.tile([P, 1], F16)
            p_keep = cpool.tile([P, NT, HID_CH], F16)
            q_keep = cpool.tile([P, NT, OUT_CH], F16)
            z_keep = cpool.tile([P, NT, OUT_CH], F16)
            logit_sb = cpool.tile([P, LCH], F32)
            nc.sync.dma_start(out=xT_s[:], in_=xT[:])
            nc.sync.dma_start(out=W1_s[:], in_=W1h[:])
            nc.sync.dma_start(out=W2_s[:], in_=W2h[:])
            nc.sync.dma_start(out=b1_s[:], in_=b1row[:])
            nc.sync.dma_start(out=b2_s[:], in_=b2row[:])
            nc.sync.dma_start(out=sq_s[:], in_=sqrow[:])
            nc.sync.dma_start(out=dk1_s[:], in_=dk1[:])
            nc.sync.dma_start(out=dk2_s[:], in_=dk2[:])
            nc.sync.dma_start(out=ei_s[:], in_=eidx[:])
            nc.sync.dma_start(out=el_s[:], in_=edloc[:])
            nc.sync.dma_start(out=li_s[:], in_=lidx[:])
            nc.vector.memset(ones_s[:], 1.0)
            nc.gpsimd.iota(iota_s[:], pattern=[[1, P]], base=0,
                           channel_multiplier=0,
                           allow_small_or_imprecise_dtypes=True)
            nc.gpsimd.iota(pcol_s[:], pattern=[[0, 1]], base=0,
                           channel_multiplier=1,
                           allow_small_or_imprecise_dtypes=True)
            pmat_s = cpool.tile([P, P], F16)
            nc.gpsimd.iota(pmat_s[:], pattern=[[0, P]], base=0,
                           channel_multiplier=1,
                           allow_small_or_imprecise_dtypes=True)
            nc.vector.tensor_scalar(
                out=ident_s[:], in0=iota_s[:], scalar1=pcol_s[:],
                scalar2=None, op0=mybir.AluOpType.is_equal)

            p_in = dram.tile([NPC, HID_CH], F16)
            PT = dram.tile([N_NODES, HID_CH], F16, addr_space="Shared")
            q_in = dram.tile([NPC, P], F16)
            QT = dram.tile([N_NODES, P], F16, addr_space="Shared")
            z_in = dram.tile([NPC, P], F16)
            ZT = dram.tile([N_NODES, P], F16, addr_space="Shared")

            # ---- stage 1: p~ = (x*dinv) @ W1, per tile; keep + publish
            for t in range(NT):
                m = min(P, NPC - t * P)
                psum_p = psB.tile([P, HID_CH], F32, tag="pp", space="PSUM")
                nc.tensor.matmul(out=psum_p[0:m, :],
                                 lhsT=xT_s[:, t * P:t * P + m],
                                 rhs=W1_s[:], start=True, stop=True)
                nc.scalar.copy(out=p_keep[0:m, t, :], in_=psum_p[0:m, :])
            nc.sync.dma_start(
                out=p_in[0:(NT - 1) * P, :].rearrange(
                    "(t p) c -> p t c", p=P),
                in_=p_keep[:, 0:NT - 1, :])
            nc.sync.dma_start(out=p_in[(NT - 1) * P:NPC, :],
                              in_=p_keep[0:NPC - (NT - 1) * P, NT - 1, :])

            nc.gpsimd.collective_compute(
                "AllGather", mybir.AluOpType.bypass, replica_groups=RG,
                ins=[p_in.opt()], outs=[PT.opt()])

            def conv_layer(TBL, out_dram, is_conv1):
                keep = q_keep if is_conv1 else z_keep
                for g in meta["groups"]:
                    nch = g["nA"] + g["nB"]
                    msg = msgp.tile([P, NCHG_MAX, P], F16, tag="msg")
                    if g["nA"]:
                        nc.gpsimd.dma_gather(
                            out_ap=msg[:, 0:g["nA"], :],
                            in_ap=TBL[0:A_LIM, :],
                            idxs_ap=ei_s[:, g["colA"]:g["colA"] + g["nA"] * 8],
                            num_idxs=g["nA"] * P, num_idxs_reg=g["nA"] * P,
                            elem_size=P, single_packet=False)
                    if g["nB"]:
                        nc.gpsimd.dma_gather(
                            out_ap=msg[:, g["nA"]:nch, :],
                            in_ap=TBL[B_OFF:N_NODES, :],
                            idxs_ap=ei_s[:, g["colB"]:g["colB"] + g["nB"] * 8],
                            num_idxs=g["nB"] * P, num_idxs_reg=g["nB"] * P,
                            elem_size=P, single_packet=False)
                    for t in g["tiles"]:
                        m = min(P, NPC - t * P)
                        chunks = meta["tile_chunks"][t]
                        if is_conv1:
                            # psum [ch, d], seeded outer(b1, sqrtdeg)
                            ps = psA.tile([HID_CH, P], F32, tag="agg1",
                                          space="PSUM")
                            nc.tensor.matmul(
                                out=ps[:, 0:m], lhsT=b1_s[:],
                                rhs=sq_s[:, t * P:t * P + m],
                                start=True, stop=False)
                        else:
                            # psum [d, ch], seeded outer(sqrtdeg, b2)
                            ps = psA.tile([P, OUT_CH], F32, tag="agg2",
                                          space="PSUM")
                            nc.tensor.matmul(
                                out=ps[0:m, :],
                                lhsT=sq_s[:, t * P:t * P + m],
                                rhs=b2_s[:], start=True, stop=False)
                        for gc in chunks:
                            lc = gc - g["base"]
                            ind = indp.tile([P, P], F16, tag="ind")
                            nc.vector.tensor_scalar(
                                out=ind[:], in0=iota_s[:],
                                scalar1=el_s[:, gc:gc + 1],
                                scalar2=None,
                                op0=mybir.AluOpType.is_equal)
                            if is_conv1:
                                nc.tensor.matmul(
                                    out=ps[:, 0:m], lhsT=msg[:, lc, :],
                                    rhs=ind[:, 0:m],
                                    start=False, stop=False)
                            else:
                                nc.tensor.matmul(
                                    out=ps[0:m, :], lhsT=ind[:, 0:m],
                                    rhs=msg[:, lc, 0:OUT_CH],
                                    start=False, stop=False)
                        # self-loop: += p~[d] (resp. q~[d]) via identity
                        if is_conv1:
                            nc.tensor.matmul(
                                out=ps[:, 0:m], lhsT=p_keep[0:m, t, :],
                                rhs=ident_s[0:m, 0:m],
                                start=False, stop=True)
                            hT = evac.tile([HID_CH, P], F16, tag="hT")
                            nc.scalar.activation(
                                out=hT[:, 0:m], in_=ps[:, 0:m],
                                func=mybir.ActivationFunctionType.Relu)
                            psq = psB.tile([P, HID_CH], F32, tag="pp",
                                           space="PSUM")
                            nc.tensor.matmul(out=psq[0:m, 0:OUT_CH],
                                             lhsT=hT[:, 0:m], rhs=W2_s[:],
                                             start=True, stop=True)
                            nc.scalar.activation(
                                out=q_keep[0:m, t, :],
                                in_=psq[0:m, 0:OUT_CH],
                                func=mybir.ActivationFunctionType.Copy,
                                scale=dk2_s[0:m, t:t + 1])
                        else:
                            nc.tensor.matmul(
                                out=ps[0:m, :], lhsT=ident_s[0:m, 0:m],
                                rhs=q_keep[0:m, t, :],
                                start=False, stop=True)
                            nc.scalar.activation(
                                out=z_keep[0:m, t, :], in_=ps[0:m, :],
                                func=mybir.ActivationFunctionType.Copy,
                                scale=dk1_s[0:m, t:t + 1])

            conv_layer(PT, q_in, True)
            nc.sync.dma_start(
                out=q_in[0:(NT - 1) * P, 0:OUT_CH].rearrange(
                    "(t p) c -> p t c", p=P),
                in_=q_keep[:, 0:NT - 1, :])
            nc.sync.dma_start(out=q_in[(NT - 1) * P:NPC, 0:OUT_CH],
                              in_=q_keep[0:NPC - (NT - 1) * P, NT - 1, :])
            nc.gpsimd.collective_compute(
                "AllGather", mybir.AluOpType.bypass, replica_groups=RG,
                ins=[q_in.opt()], outs=[QT.opt()])
            conv_layer(QT, z_in, False)
            nc.sync.dma_start(
                out=z_in[0:(NT - 1) * P, 0:OUT_CH].rearrange(
                    "(t p) c -> p t c", p=P),
                in_=z_keep[:, 0:NT - 1, :])
            nc.sync.dma_start(out=z_in[(NT - 1) * P:NPC, 0:OUT_CH],
                              in_=z_keep[0:NPC - (NT - 1) * P, NT - 1, :])
            nc.gpsimd.collective_compute(
                "AllGather", mybir.AluOpType.bypass, replica_groups=RG,
                ins=[z_in.opt()], outs=[ZT.opt()])

            # ---- decode: za via PE selection from z_keep, zb via
            # transposed dma_gather from ZT; dot = DVE mult + PE reduce
            for b in meta["lbatches"]:
                nch = b["nch"]
                zbT = decp.tile([P, LBATCH * P], F16, tag="zbT")
                av = (0, A_LIM) if b["view"] == 0 else (B_OFF, N_NODES)
                nc.gpsimd.dma_gather(
                    out_ap=zbT[:, 0:nch * P].rearrange(
                        "p (a b) -> p a b", a=1),
                    in_ap=ZT[av[0]:av[1], :],
                    idxs_ap=li_s[:, b["base"] * 8:(b["base"] + nch) * 8],
                    num_idxs=nch * P, num_idxs_reg=nch * P,
                    elem_size=P, single_packet=False, transpose=True)
                # ACT firewall: don't let DVE read dma_gather-written SBUF
                zb2 = decp.tile([P, LBATCH * P], F16, tag="zb2")
                nc.scalar.copy(out=zb2[0:OUT_CH, 0:nch * P],
                               in_=zbT[0:OUT_CH, 0:nch * P])
                la_t = decp.tile([P, LBATCH * P], F16, tag="la")
                nc.sync.dma_start(
                    out=la_t[:, 0:nch * P],
                    in_=laloc[:, b["base"] * P:(b["base"] + nch) * P])
                for ci in range(nch):
                    t = b["tiles"][ci]
                    m = min(P, NPC - t * P)
                    sel = indp.tile([P, P], F16, tag="sel")
                    nc.vector.tensor_tensor(
                        out=sel[:], in0=pmat_s[:],
                        in1=la_t[:, ci * P:ci * P + P],
                        op=mybir.AluOpType.is_equal)
                    psa = psA.tile([HID_CH, P], F32, tag="agg1",
                                   space="PSUM")
                    nc.tensor.matmul(out=psa[0:OUT_CH, :],
                                     lhsT=z_keep[0:m, t, :],
                                     rhs=sel[0:m, :], start=True, stop=True)
                    scr = indp.tile([OUT_CH, P], F16, tag="scr")
                    nc.vector.tensor_tensor(
                        out=scr[:], in0=psa[0:OUT_CH, :],
                        in1=zb2[0:OUT_CH, ci * P:ci * P + P],
                        op=mybir.AluOpType.mult)
                    psl = psB.tile([P, HID_CH], F32, tag="pp",
                                   space="PSUM")
                    nc.tensor.matmul(out=psl[:, 0:1], lhsT=scr[:],
                                     rhs=ones_s[0:OUT_CH, :],
                                     start=True, stop=True)
                    cc = b["base"] + ci
                    nc.scalar.copy(out=logit_sb[:, cc:cc + 1], in_=psl[:, 0:1])
            nc.sync.dma_start(out=logits[:], in_=logit_sb[:])

    nc.compile()
    return nc


_CACHE = {}
TRACE = False          # set True (e.g. from test.py) to capture an NTFF trace
LAST_RESULT = None     # BassKernelResults of the most recent run


def kernel(**inputs):
    meta, cores, perms = _prepare(**inputs)
    key = (meta["TOT_CH"], meta["LCH"], meta["WC"], meta["WL"])
    if key not in _CACHE:
        _CACHE[key] = _build(meta)
    nc = _CACHE[key]
    names = ("xT", "W1h", "W2h", "b1row", "b2row", "sqrow", "dk1", "dk2",
             "eidx", "edloc", "lidx", "laloc")
    in_maps = [{n: c[n] for n in names} for c in cores]
    res = run_bass_kernel_spmd(nc, in_maps, core_ids=list(range(N_CORES)),
                               trace=TRACE)
    global LAST_RESULT
    LAST_RESULT = res
    out = np.empty(N_LABEL, np.float32)
    for k in range(N_CORES):
        vals = res.results[k]["logits"].T.ravel()
        perm = perms[k]
        m = perm >= 0
        out[perm[m]] = vals[m]
    return out


# revision 20
# speedup vs baseline: 1.2208x; 1.0048x over previous
"""GCN link predictor on 8 TRN2 NeuronCores (Bass/Tile) — v2.

Design notes (driven by HW profiling of the v1 baseline, 2.34 ms):
the bottleneck is the GPSIMD (Pool) engine generating SWDGE descriptors for
dma_gather at ~8 ns/index, blocking, with no faster indexed primitive on the
machine (ap_gather ~27 ns/idx, scatter_add wedges the device).  So v2
minimizes gather indices and keeps everything else off the Pool engine:

- Normalization refactor: out[d] = b + dinv[d] * sum_e table[src_e] with
  table rows pre-scaled by dinv[src] (x rows host-scaled; q~ scaled dinv^2
  at evacuation).  Indicators become PURE one-hot -> single-op is_equal on
  DVE, and PSUM is seeded with outer(bias, sqrt(deg)) so the dst-side dinv
  folds into the existing evacuation scale.
- Self-loops never enter the gather path: one identity matmul per dst tile
  adds p~[d] (resp. q~[d]) from SBUF-resident local tiles.
- Decode: label pairs sharded by core(a); the a-side z rows come from PE
  one-hot selection out of SBUF-resident local z tiles (overlaps the last
  AllGather); only the b-side uses dma_gather (transpose=True -> zbT
  [ch, pair]); dot product = DVE multiply + PE ones-reduction.

dma_gather indices are int16, so 40000-row tables are addressed through two
overlapping views: A = rows [0, 32768), B = rows [7232, 40000).
"""

import numpy as np

import concourse.bass as bass
import concourse.bacc as bacc
import concourse.mybir as mybir
import concourse.tile as tile
from concourse.bass_utils import run_bass_kernel_spmd

P = 128
N_NODES = 40000
IN_CH = 128
HID_CH = 128
OUT_CH = 64
N_LABEL = 200000
N_CORES = 8
NPC = N_NODES // N_CORES          # 5000 nodes per core
NT = (NPC + P - 1) // P           # 40 dst tiles per core (last has 8 nodes)
A_LIM = 32768                     # view A = rows [0, 32768)
B_OFF = N_NODES - A_LIM           # 7232; view B = rows [7232, 40000)
GROUP = 4                         # dst tiles per gather group
LBATCH = 32                       # decode chunks per gather batch

F16 = mybir.dt.float16
F32 = mybir.dt.float32
I16 = mybir.dt.int16


def _wrap16(flat):
    """dma_gather / index SBUF image: position n -> [n % 16, n // 16],
    replicated across the 8 groups of 16 partitions. [128, len/16] int16."""
    n = len(flat)
    assert n % 16 == 0
    grid = np.asarray(flat, np.int16).reshape(n // 16, 16).T
    return np.tile(grid, (8, 1))


def _prepare(x, edge_index, edge_label_index, W1, b1, W2, b2):
    src = np.asarray(edge_index[0], np.int64)
    dst = np.asarray(edge_index[1], np.int64)
    deg = (np.bincount(dst, minlength=N_NODES) + 1).astype(np.float64)
    dinv = (1.0 / np.sqrt(deg)).astype(np.float32)
    sqrtdeg = np.sqrt(deg).astype(np.float32)

    # ---- bucket edges by (core, tile), sorted by src within each bucket
    core_of = dst // NPC
    tloc = (dst % NPC) // P
    order = np.lexsort((src, tloc, core_of))
    s_src = src[order]
    s_dst = dst[order]
    key = core_of[order] * NT + tloc[order]
    starts = np.searchsorted(key, np.arange(N_CORES * NT))
    ends = np.searchsorted(key, np.arange(N_CORES * NT) + 1)

    cnt = (ends - starts).reshape(N_CORES, NT)
    fA = np.empty((N_CORES, NT), np.int64)   # forced-A (< B_OFF)
    fB = np.empty((N_CORES, NT), np.int64)   # forced-B (>= A_LIM)
    for k in range(N_CORES):
        for t in range(NT):
            b = k * NT + t
            ss = s_src[starts[b]:ends[b]]
            fA[k, t] = np.searchsorted(ss, B_OFF)
            fB[k, t] = len(ss) - np.searchsorted(ss, A_LIM)
    NCA = np.maximum(1, (fA.max(axis=0) + P - 1) // P)          # [NT]
    nA = np.minimum(cnt - fB, P * NCA[None, :])                 # [cores, NT]
    nA = np.maximum(nA, 0)
    cntB = cnt - nA
    NCB = (cntB.max(axis=0) + P - 1) // P                       # [NT]

    groups = []
    tile_chunks = {}
    gbase = 0
    col = 0
    for g0 in range(0, NT, GROUP):
        ts = list(range(g0, min(g0 + GROUP, NT)))
        gnA = int(NCA[ts].sum())
        gnB = int(NCB[ts].sum())
        groups.append(dict(tiles=ts, base=gbase, nA=gnA, nB=gnB,
                           colA=col, colB=col + gnA * 8))
        ca = gbase
        cb = gbase + gnA
        for t in ts:
            tile_chunks[t] = (list(range(ca, ca + int(NCA[t])))
                              + list(range(cb, cb + int(NCB[t]))))
            ca += int(NCA[t])
            cb += int(NCB[t])
        gbase += gnA + gnB
        col += (gnA + gnB) * 8
    TOT_CH = gbase
    WC = col

    # ---- per-core conv arrays: gather indices + dst-local one-hot columns
    cores = []
    for k in range(N_CORES):
        eidx = np.zeros((P, WC), np.int16)
        edloc = np.full((P, TOT_CH), -1.0, np.float32)  # -1 -> no is_eq match
        for g in groups:
            flatA = []
            flatB = []
            for t in g["tiles"]:
                b = k * NT + t
                ss = s_src[starts[b]:ends[b]]
                dd = s_dst[starts[b]:ends[b]]
                na = int(nA[k, t])
                la = np.full(int(NCA[t]) * P, -1.0, np.float32)
                ia = np.zeros(int(NCA[t]) * P, np.int64)
                ia[:na] = ss[:na]
                la[:na] = (dd[:na] - k * NPC - t * P).astype(np.float32)
                lb_ = np.full(int(NCB[t]) * P, -1.0, np.float32)
                ib = np.zeros(int(NCB[t]) * P, np.int64)
                nb = int(cntB[k, t])
                ib[:nb] = ss[na:na + nb] - B_OFF
                lb_[:nb] = (dd[na:na + nb] - k * NPC - t * P).astype(
                    np.float32)
                flatA.append((ia, la))
                flatB.append((ib, lb_))
            ia = np.concatenate([f[0] for f in flatA])
            ib = np.concatenate([f[0] for f in flatB])
            locs = np.concatenate([f[1] for f in flatA]
                                  + [f[1] for f in flatB])
            if len(ia):
                eidx[:, g["colA"]:g["colA"] + len(ia) // 16] = _wrap16(ia)
            if len(ib):
                eidx[:, g["colB"]:g["colB"] + len(ib) // 16] = _wrap16(ib)
            nch = g["nA"] + g["nB"]
            edloc[:, g["base"]:g["base"] + nch] = locs.reshape(nch, P).T
        cores.append(dict(eidx=eidx, edloc=edloc))

    # ---- decode prep: pairs sharded by core(a), grouped by (a_tile, b_view)
    la_all = np.asarray(edge_label_index[0], np.int64)
    lb_all = np.asarray(edge_label_index[1], np.int64)
    owner = la_all // NPC
    # per (core, a_tile, view) counts to find shared padded chunk counts
    atile = (la_all % NPC) // P
    bview = (lb_all >= A_LIM).astype(np.int64)  # 0 -> A view, 1 -> B view
    cntd = np.zeros((N_CORES, NT, 2), np.int64)
    for k in range(N_CORES):
        m = owner == k
        np.add.at(cntd[k], (atile[m], bview[m]), 1)
    NCD = (cntd.max(axis=0) + P - 1) // P                       # [NT, 2]
    # chunk layout: all view-A chunks (tile-major), then all view-B chunks
    chunksA = []
    chunksB = []
    for t in range(NT):
        for c in range(int(NCD[t, 0])):
            chunksA.append(t)
        for c in range(int(NCD[t, 1])):
            chunksB.append(t)
    LCH = len(chunksA) + len(chunksB)
    lbatches = []
    for v, chs, base in ((0, chunksA, 0), (1, chunksB, len(chunksA))):
        c0 = 0
        while c0 < len(chs):
            rem = len(chs) - c0
            # keep the final batches small so the post-AG3 tail is short
            nch = min(LBATCH if rem > LBATCH else max(8, rem // 2), rem)
            lbatches.append(dict(view=v, base=base + c0, nch=nch,
                                 tiles=chs[c0:c0 + nch]))
            c0 += nch
    WL = LCH * 8

    perms = []
    for k in range(N_CORES):
        m = owner == k
        ga, gb, gidx = la_all[m], lb_all[m], np.nonzero(m)[0]
        at, bv = atile[m], bview[m]
        o = np.lexsort((gb, bv, at))
        ga, gb, gidx, at, bv = ga[o], gb[o], gidx[o], at[o], bv[o]
        lidx = np.zeros((P, WL), np.int16)
        laloc = np.full(LCH * P, -1.0, np.float32)
        perm = np.full(LCH * P, -1, np.int64)
        cbase = {0: 0, 1: len(chunksA)}
        coff = {0: 0, 1: 0}
        for t in range(NT):
            for v in (0, 1):
                mm = (at == t) & (bv == v)
                pa, pb, pi = ga[mm], gb[mm], gidx[mm]
                ncap = int(NCD[t, v]) * P
                assert len(pa) <= ncap
                ids = np.zeros(ncap, np.int64)
                ids[:len(pb)] = pb - (0 if v == 0 else B_OFF)
                start = cbase[v] + coff[v]
                lidx[:, start * 8:(start + int(NCD[t, v])) * 8] = \
                    _wrap16(ids)
                sl = slice(start * P, start * P + len(pa))
                laloc[start * P:(start + int(NCD[t, v])) * P][:len(pa)] = \
                    (pa - k * NPC - t * P).astype(np.float32)
                perm[sl] = pi
                coff[v] += int(NCD[t, v])
        # laloc broadcast image: [128 partitions, LCH*128] fp16, value =
        # a_loc of the pair in that column (same in every partition)
        lab = np.broadcast_to(laloc[None, :], (P, LCH * P)).astype(np.float16)
        cores[k]["lidx"] = lidx
        cores[k]["laloc"] = np.ascontiguousarray(lab)
        perms.append(perm)

    # ---- dense inputs per core
    x = np.asarray(x, np.float32)
    for k in range(N_CORES):
        xk = x[k * NPC:(k + 1) * NPC] * dinv[k * NPC:(k + 1) * NPC, None]
        cores[k]["xT"] = np.ascontiguousarray(xk.T).astype(np.float16)
        cores[k]["W1h"] = np.asarray(W1, np.float32).astype(np.float16)
        cores[k]["W2h"] = np.asarray(W2, np.float32).astype(np.float16)
        cores[k]["b1row"] = np.asarray(b1, np.float32).astype(
            np.float16).reshape(1, HID_CH)
        cores[k]["b2row"] = np.asarray(b2, np.float32).astype(
            np.float16).reshape(1, OUT_CH)
        sq = np.zeros((1, NT * P), np.float16)
        sq[0, :NPC] = sqrtdeg[k * NPC:(k + 1) * NPC]
        cores[k]["sqrow"] = sq
        dk1 = np.ones((P, NT), np.float32)
        dk2 = np.ones((P, NT), np.float32)
        dv = dinv[k * NPC:(k + 1) * NPC]
        for t in range(NT):
            m = min(P, NPC - t * P)
            dk1[:m, t] = dv[t * P:t * P + m]
            dk2[:m, t] = dv[t * P:t * P + m] ** 2
        cores[k]["dk1"] = dk1
        cores[k]["dk2"] = dk2

    meta = dict(groups=groups, tile_chunks=tile_chunks, TOT_CH=TOT_CH,
                WC=WC, lbatches=lbatches, LCH=LCH, WL=WL,
                NCD=[[int(v) for v in row] for row in NCD])
    return meta, cores, perms


def _build(meta):
    TOT_CH, WC, LCH, WL = (meta["TOT_CH"], meta["WC"],
                           meta["LCH"], meta["WL"])
    NCHG_MAX = max(g["nA"] + g["nB"] for g in meta["groups"])

    nc = bacc.Bacc("TRN2", target_bir_lowering=False, debug=False,
                   num_devices=N_CORES)
    xT = nc.dram_tensor("xT", [P, NPC], F16, kind="ExternalInput")
    W1h = nc.dram_tensor("W1h", [P, HID_CH], F16, kind="ExternalInput")
    W2h = nc.dram_tensor("W2h", [P, OUT_CH], F16, kind="ExternalInput")
    b1row = nc.dram_tensor("b1row", [1, HID_CH], F16, kind="ExternalInput")
    b2row = nc.dram_tensor("b2row", [1, OUT_CH], F16, kind="ExternalInput")
    sqrow = nc.dram_tensor("sqrow", [1, NT * P], F16, kind="ExternalInput")
    dk1 = nc.dram_tensor("dk1", [P, NT], F32, kind="ExternalInput")
    dk2 = nc.dram_tensor("dk2", [P, NT], F32, kind="ExternalInput")
    eidx = nc.dram_tensor("eidx", [P, WC], I16, kind="ExternalInput")
    edloc = nc.dram_tensor("edloc", [P, TOT_CH], F32, kind="ExternalInput")
    lidx = nc.dram_tensor("lidx", [P, WL], I16, kind="ExternalInput")
    laloc = nc.dram_tensor("laloc", [P, LCH * P], F16, kind="ExternalInput")
    logits = nc.dram_tensor("logits", [P, LCH], F32, kind="ExternalOutput")

    RG = [list(range(N_CORES))]

    with tile.TileContext(nc) as tc:
        with tc.tile_pool(name="const", bufs=1) as cpool, \
             tc.tile_pool(name="msgp", bufs=4) as msgp, \
             tc.tile_pool(name="indp", bufs=8) as indp, \
             tc.tile_pool(name="evac", bufs=4) as evac, \
             tc.tile_pool(name="decp", bufs=3) as decp, \
             tc.tile_pool(name="psA", bufs=3, space="PSUM") as psA, \
             tc.tile_pool(name="psB", bufs=2, space="PSUM") as psB, \
             tc.tile_pool(name="dram", bufs=1, space="DRAM") as dram:

            # constants into SBUF
            xT_s = cpool.tile([P, NPC], F16)
            W1_s = cpool.tile([P, HID_CH], F16)
            W2_s = cpool.tile([P, OUT_CH], F16)
            b1_s = cpool.tile([1, HID_CH], F16)
            b2_s = cpool.tile([1, OUT_CH], F16)
            sq_s = cpool.tile([1, NT * P], F16)
            dk1_s = cpool.tile([P, NT], F32)
            dk2_s = cpool.tile([P, NT], F32)
            ei_s = cpool.tile([P, WC], I16)
            el_s = cpool.tile([P, TOT_CH], F32)
            li_s = cpool.tile([P, WL], I16)
            iota_s = cpool.tile([P, P], F16)
            pcol_s = cpool.tile([P, 1], F32)
            ident_s = cpool.tile([P, P], F16)
            ones_s = cpool.tile([P, 1], F16)
            p_keep = cpool.tile([P, NT, HID_CH], F16)
            q_keep = cpool.tile([P, NT, OUT_CH], F16)
            z_keep = cpool.tile([P, NT, OUT_CH], F16)
            logit_sb = cpool.tile([P, LCH], F32)
            nc.sync.dma_start(out=xT_s[:], in_=xT[:])
            nc.sync.dma_start(out=W1_s[:], in_=W1h[:])
            nc.sync.dma_start(out=W2_s[:], in_=W2h[:])
            nc.sync.dma_start(out=b1_s[:], in_=b1row[:])
            nc.sync.dma_start(out=b2_s[:], in_=b2row[:])
            nc.sync.dma_start(out=sq_s[:], in_=sqrow[:])
            nc.sync.dma_start(out=dk1_s[:], in_=dk1[:])
            nc.sync.dma_start(out=dk2_s[:], in_=dk2[:])
            nc.sync.dma_start(out=ei_s[:], in_=eidx[:])
            nc.sync.dma_start(out=el_s[:], in_=edloc[:])
            nc.sync.dma_start(out=li_s[:], in_=lidx[:])
            nc.vector.memset(ones_s[:], 1.0)
            nc.gpsimd.iota(iota_s[:], pattern=[[1, P]], base=0,
                           channel_multiplier=0,
                           allow_small_or_imprecise_dtypes=True)
            nc.gpsimd.iota(pcol_s[:], pattern=[[0, 1]], base=0,
                           channel_multiplier=1,
                           allow_small_or_imprecise_dtypes=True)
            pmat_s = cpool.tile([P, P], F16)
            nc.gpsimd.iota(pmat_s[:], pattern=[[0, P]], base=0,
                           channel_multiplier=1,
                           allow_small_or_imprecise_dtypes=True)
            nc.vector.tensor_scalar(
                out=ident_s[:], in0=iota_s[:], scalar1=pcol_s[:],
                scalar2=None, op0=mybir.AluOpType.is_equal)

            p_in = dram.tile([NPC, HID_CH], F16)
            PT = dram.tile([N_NODES, HID_CH], F16, addr_space="Shared")
            q_in = dram.tile([NPC, P], F16)
            QT = dram.tile([N_NODES, P], F16, addr_space="Shared")
            z_in = dram.tile([NPC, P], F16)
            ZT = dram.tile([N_NODES, P], F16, addr_space="Shared")

            # ---- stage 1: p~ = (x*dinv) @ W1, per tile; keep + publish
            for t in range(NT):
                m = min(P, NPC - t * P)
                psum_p = psB.tile([P, HID_CH], F32, tag="pp", space="PSUM")
                nc.tensor.matmul(out=psum_p[0:m, :],
                                 lhsT=xT_s[:, t * P:t * P + m],
                                 rhs=W1_s[:], start=True, stop=True)
                nc.scalar.copy(out=p_keep[0:m, t, :], in_=psum_p[0:m, :])
            nc.sync.dma_start(
                out=p_in[0:(NT - 1) * P, :].rearrange(
                    "(t p) c -> p t c", p=P),
                in_=p_keep[:, 0:NT - 1, :])
            nc.sync.dma_start(out=p_in[(NT - 1) * P:NPC, :],
                              in_=p_keep[0:NPC - (NT - 1) * P, NT - 1, :])

            nc.gpsimd.collective_compute(
                "AllGather", mybir.AluOpType.bypass, replica_groups=RG,
                ins=[p_in.opt()], outs=[PT.opt()])

            def conv_layer(TBL, out_dram, is_conv1):
                keep = q_keep if is_conv1 else z_keep
                for g in meta["groups"]:
                    nch = g["nA"] + g["nB"]
                    msg = msgp.tile([P, NCHG_MAX, P], F16, tag="msg")
                    if g["nA"]:
                        nc.gpsimd.dma_gather(
                            out_ap=msg[:, 0:g["nA"], :],
                            in_ap=TBL[0:A_LIM, :],
                            idxs_ap=ei_s[:, g["colA"]:g["colA"] + g["nA"] * 8],
                            num_idxs=g["nA"] * P, num_idxs_reg=g["nA"] * P,
                            elem_size=P, single_packet=False)
                    if g["nB"]:
                        nc.gpsimd.dma_gather(
                            out_ap=msg[:, g["nA"]:nch, :],
                            in_ap=TBL[B_OFF:N_NODES, :],
                            idxs_ap=ei_s[:, g["colB"]:g["colB"] + g["nB"] * 8],
                            num_idxs=g["nB"] * P, num_idxs_reg=g["nB"] * P,
                            elem_size=P, single_packet=False)
                    for t in g["tiles"]:
                        m = min(P, NPC - t * P)
                        chunks = meta["tile_chunks"][t]
                        if is_conv1:
                            # psum [ch, d], seeded outer(b1, sqrtdeg)
                            ps = psA.tile([HID_CH, P], F32, tag="agg1",
                                          space="PSUM")
                            nc.tensor.matmul(
                                out=ps[:, 0:m], lhsT=b1_s[:],
                                rhs=sq_s[:, t * P:t * P + m],
                                start=True, stop=False)
                        else:
                            # psum [d, ch], seeded outer(sqrtdeg, b2)
                            ps = psA.tile([P, OUT_CH], F32, tag="agg2",
                                          space="PSUM")
                            nc.tensor.matmul(
                                out=ps[0:m, :],
                                lhsT=sq_s[:, t * P:t * P + m],
                                rhs=b2_s[:], start=True, stop=False)
                        for gc in chunks:
                            lc = gc - g["base"]
                            ind = indp.tile([P, P], F16, tag="ind")
                            nc.vector.tensor_scalar(
                                out=ind[:], in0=iota_s[:],
                                scalar1=el_s[:, gc:gc + 1],
                                scalar2=None,
                                op0=mybir.AluOpType.is_equal)
                            if is_conv1:
                                nc.tensor.matmul(
                                    out=ps[:, 0:m], lhsT=msg[:, lc, :],
                                    rhs=ind[:, 0:m],
                                    start=False, stop=False)
                            else:
                                nc.tensor.matmul(
                                    out=ps[0:m, :], lhsT=ind[:, 0:m],
                                    rhs=msg[:, lc, 0:OUT_CH],
                                    start=False, stop=False)
                        # self-loop: += p~[d] (resp. q~[d]) via identity
                        if is_conv1:
                            nc.tensor.matmul(
                                out=ps[:, 0:m], lhsT=p_keep[0:m, t, :],
                                rhs=ident_s[0:m, 0:m],
                                start=False, stop=True)
                            hT = evac.tile([HID_CH, P], F16, tag="hT")
                            nc.scalar.activation(
                                out=hT[:, 0:m], in_=ps[:, 0:m],
                                func=mybir.ActivationFunctionType.Relu)
                            psq = psB.tile([P, HID_CH], F32, tag="pp",
                                           space="PSUM")
                            nc.tensor.matmul(out=psq[0:m, 0:OUT_CH],
                                             lhsT=hT[:, 0:m], rhs=W2_s[:],
                                             start=True, stop=True)
                            nc.scalar.activation(
                                out=q_keep[0:m, t, :],
                                in_=psq[0:m, 0:OUT_CH],
                                func=mybir.ActivationFunctionType.Copy,
                                scale=dk2_s[0:m, t:t + 1])
                        else:
                            nc.tensor.matmul(
                                out=ps[0:m, :], lhsT=ident_s[0:m, 0:m],
                                rhs=q_keep[0:m, t, :],
                                start=False, stop=True)
                            nc.scalar.activation(
                                out=z_keep[0:m, t, :], in_=ps[0:m, :],
                                func=mybir.ActivationFunctionType.Copy,
                                scale=dk1_s[0:m, t:t + 1])

            conv_layer(PT, q_in, True)
            nc.sync.dma_start(
                out=q_in[0:(NT - 1) * P, 0:OUT_CH].rearrange(
                    "(t p) c -> p t c", p=P),
                in_=q_keep[:, 0:NT - 1, :])
            nc.sync.dma_start(out=q_in[(NT - 1) * P:NPC, 0:OUT_CH],
                              in_=q_keep[0:NPC - (NT - 1) * P, NT - 1, :])
            nc.gpsimd.collective_compute(
                "AllGather", mybir.AluOpType.bypass, replica_groups=RG,
                ins=[q_in.opt()], outs=[QT.opt()])
            conv_layer(QT, z_in, False)
            nc.sync.dma_start(
                out=z_in[0:(NT - 1) * P, 0:OUT_CH].rearrange(
                    "(t p) c -> p t c", p=P),
                in_=z_keep[:, 0:NT - 1, :])
            nc.sync.dma_start(out=z_in[(NT - 1) * P:NPC, 0:OUT_CH],
                              in_=z_keep[0:NPC - (NT - 1) * P, NT - 1, :])
            nc.gpsimd.collective_compute(
                "AllGather", mybir.AluOpType.bypass, replica_groups=RG,
                ins=[z_in.opt()], outs=[ZT.opt()])

            # ---- decode: za via PE selection from z_keep, zb via
            # transposed dma_gather from ZT; dot = DVE mult + PE reduce
            for b in meta["lbatches"]:
                nch = b["nch"]
                zbT = decp.tile([P, LBATCH * P], F16, tag="zbT")
                av = (0, A_LIM) if b["view"] == 0 else (B_OFF, N_NODES)
                nc.gpsimd.dma_gather(
                    out_ap=zbT[:, 0:nch * P].rearrange(
                        "p (a b) -> p a b", a=1),
                    in_ap=ZT[av[0]:av[1], :],
                    idxs_ap=li_s[:, b["base"] * 8:(b["base"] + nch) * 8],
                    num_idxs=nch * P, num_idxs_reg=nch * P,
                    elem_size=P, single_packet=False, transpose=True)
                # ACT firewall: don't let DVE read dma_gather-written SBUF
                zb2 = decp.tile([P, LBATCH * P], F16, tag="zb2")
                nc.scalar.copy(out=zb2[0:OUT_CH, 0:nch * P],
                               in_=zbT[0:OUT_CH, 0:nch * P])
                la_t = decp.tile([P, LBATCH * P], F16, tag="la")
                nc.sync.dma_start(
                    out=la_t[:, 0:nch * P],
                    in_=laloc[:, b["base"] * P:(b["base"] + nch) * P])
                for ci in range(nch):
                    t = b["tiles"][ci]
                    m = min(P, NPC - t * P)
                    sel = indp.tile([P, P], F16, tag="sel")
                    nc.vector.tensor_tensor(
                        out=sel[:], in0=pmat_s[:],
                        in1=la_t[:, ci * P:ci * P + P],
                        op=mybir.AluOpType.is_equal)
                    psa = psA.tile([HID_CH, P], F32, tag="agg1",
                                   space="PSUM")
                    nc.tensor.matmul(out=psa[0:OUT_CH, :],
                                     lhsT=z_keep[0:m, t, :],
                                     rhs=sel[0:m, :], start=True, stop=True)
                    scr = indp.tile([OUT_CH, P], F16, tag="scr")
                    nc.vector.tensor_tensor(
                        out=scr[:], in0=psa[0:OUT_CH, :],
                        in1=zb2[0:OUT_CH, ci * P:ci * P + P],
                        op=mybir.AluOpType.mult)
                    psl = psB.tile([P, HID_CH], F32, tag="pp",
                                   space="PSUM")
                    nc.tensor.matmul(out=psl[:, 0:1], lhsT=scr[:],
                                     rhs=ones_s[0:OUT_CH, :],
                                     start=True, stop=True)
                    cc = b["base"] + ci
                    nc.scalar.copy(out=logit_sb[:, cc:cc + 1], in_=psl[:, 0:1])
            nc.sync.dma_start(out=logits[:], in_=logit_sb[:])

    nc.compile()
    return nc


_CACHE = {}
TRACE = False          # set True (e.g. from test.py) to capture an NTFF trace
LAST_RESULT = None     # BassKernelResults of the most recent run


def kernel(**inputs):
    meta, cores, perms = _prepare(**inputs)
    key = (meta["TOT_CH"], meta["LCH"], meta["WC"], meta["WL"])
    if key not in _CACHE:
        _CACHE[key] = _build(meta)
    nc = _CACHE[key]
    names = ("xT", "W1h", "W2h", "b1row", "b2row", "sqrow", "dk1", "dk2",
             "eidx", "edloc", "lidx", "laloc")
    in_maps = [{n: c[n] for n in names} for c in cores]
    res = run_bass_kernel_spmd(nc, in_maps, core_ids=list(range(N_CORES)),
                               trace=TRACE)
    global LAST_RESULT
    LAST_RESULT = res
    out = np.empty(N_LABEL, np.float32)
    for k in range(N_CORES):
        vals = res.results[k]["logits"].T.ravel()
        perm = perms[k]
        m = perm >= 0
        out[perm[m]] = vals[m]
    return out
